# revision 15
# baseline (speedup 1.0000x reference)
"""Distributed GQA attention block (dense transformer) on 8 TRN2 NeuronCores.

Strategy: tensor-parallel over heads. Each core owns 4 query heads + 1 KV head
(GQA group). x^T is replicated; Q/K/V projections, RoPE, scores, softmax and
the attention output all stay in "transposed" layout (feature dim on SBUF
partitions, sequence on the free dim) so no on-device transposes are needed.
The per-core attention outputs are exchanged with a single AllToAll (each core
keeps a 256-row slice of the sequence), then each core computes its slice of
the output projection against the full (replicated) wo. Host concatenates the
8 row slices.

All matmuls run in bf16 with fp32 PSUM accumulation; softmax exp runs in fp32
on the scalar engine (no max-subtraction needed: |scores*scale| <~ 12).
"""

import numpy as np
import ml_dtypes

import concourse.bass as bass
import concourse.mybir as mybir
import concourse.tile as tile
from concourse import bacc
from concourse import bass_utils

F32 = mybir.dt.float32
BF16 = mybir.dt.bfloat16

# Problem shape (hardcoded per harness contract).
L = 2048          # sequence length
D = 4096          # model dim
DH = 128          # head dim
NHEADS = 32
NKV = 8
NCORES = 8
HQ = NHEADS // NCORES      # 4 query heads per core
ROPE_THETA = 10000.0
SCALE = DH ** -0.5

ND = D // 128              # 32 contraction chunks over model dim
NLC = L // 512             # 4 free-dim chunks of 512 over sequence
NJ = L // 128              # 16 key chunks of 128
NI = L // 512              # 4 query chunks of 512
IS = L // NCORES           # 256: per-core output row slice

_cached = {}


def build_kernel(debug=False):
    nc = bacc.Bacc(num_devices=NCORES)

    xT = nc.dram_tensor("xT", [D, L], BF16, kind="ExternalInput")
    # 6 head-slots (q0..q3, k, v), each pre-tiled to [128 partitions, 32*128]
    # where [:, k*128:(k+1)*128] is the [dim-chunk k] x [head col] block.
    wqkv = nc.dram_tensor("wqkv", [6 * 128, ND * 128], BF16, kind="ExternalInput")
    # wo pre-tiled: row (do*32+hd) holds the flattened [128, 512] block
    # wo[hd*128:(hd+1)*128, do*512:(do+1)*512].
    wo = nc.dram_tensor("wo", [8 * ND, 128 * 512], BF16, kind="ExternalInput")
    cosT = nc.dram_tensor("cosT", [128, L], F32, kind="ExternalInput")
    sinT = nc.dram_tensor("sinT", [128, L], F32, kind="ExternalInput")  # sign-folded
    out = nc.dram_tensor("out", [IS, D], F32, kind="ExternalOutput")
    if debug:
        dbg_qk = nc.dram_tensor("dbg_qk", [5 * 128, L], BF16, kind="ExternalOutput")
        dbg_v = nc.dram_tensor("dbg_v", [NJ * 128, DH], BF16, kind="ExternalOutput")
        dbg_e = nc.dram_tensor("dbg_e", [2 * 128, L], BF16, kind="ExternalOutput")
        dbg_rc = nc.dram_tensor("dbg_rc", [NI, 512], F32, kind="ExternalOutput")
        dbg_ot = nc.dram_tensor("dbg_ot", [128, L], BF16, kind="ExternalOutput")
        dbg_og = nc.dram_tensor("dbg_og", [NCORES * HQ * DH, IS], BF16, kind="ExternalOutput")
        dbg_send = nc.dram_tensor("dbg_send", [NCORES * HQ * DH, IS], BF16, kind="ExternalOutput")
        dbg_recv = nc.dram_tensor("dbg_recv", [NCORES * HQ * DH, IS], BF16, kind="ExternalOutput")

    swap_mask = []
    for i in range(16):
        swap_mask += [2 * i + 1, 2 * i]

    with tile.TileContext(nc) as tc:
        with (
            tc.tile_pool(name="const", bufs=1) as cpool,
            tc.tile_pool(name="persist", bufs=1) as ppool,
            tc.tile_pool(name="dram", bufs=1, space="DRAM") as dram,
        ):
            cos_sb = cpool.tile([128, L], F32)
            sin_sb = cpool.tile([128, L], F32)
            nc.sync.dma_start(cos_sb[:], cosT[:])
            nc.sync.dma_start(sin_sb[:], sinT[:])
            ones_col = cpool.tile([128, 1], BF16)
            ones_row = cpool.tile([1, 128], F32)
            nc.vector.memset(ones_col[:], 1.0)
            nc.vector.memset(ones_row[:], 1.0)

            # Roped Q^T (4 heads) + roped K^T, bf16, [head_dim=128, L]
            qk_rope = [ppool.tile([128, L], BF16, name=f"qkrope{h}") for h in range(5)]
            # V in [seq, head_dim] layout: 16 chunks of [128, 128]
            v_sb = [ppool.tile([128, DH], BF16, name=f"vsb{j}") for j in range(NJ)]

            # ---------------- Phase 1: projections + rope ----------------
            with (
                tc.tile_pool(name="wq", bufs=1) as wpool,
                tc.tile_pool(name="xt", bufs=34) as xtpool,
                tc.tile_pool(name="p1psum", bufs=1, space="PSUM") as p1ps,
                tc.tile_pool(name="ropework", bufs=3) as rwork,
            ):
                w_sb = []
                for h in range(6):
                    wt = wpool.tile([128, ND * 128], BF16, name=f"w{h}")
                    nc.sync.dma_start(wt[:], wqkv[h * 128:(h + 1) * 128, :])
                    w_sb.append(wt)

                for lc in range(NLC):
                    lsl = bass.ts(lc, 512)
                    proj_ps = [
                        p1ps.tile([128, 512], F32, tag=f"proj{h}", name=f"proj{h}_{lc}")
                        for h in range(5)
                    ]
                    xts = []
                    for dc in range(ND):
                        xt_t = xtpool.tile([128, 512], BF16, tag="xt", name=f"xt{dc}_{lc}")
                        nc.sync.dma_start(xt_t[:], xT[dc * 128:(dc + 1) * 128, lsl])
                        xts.append(xt_t)
                        for h in range(5):
                            nc.tensor.matmul(
                                proj_ps[h][:],
                                w_sb[h][:, bass.ts(dc, 128)],
                                xt_t[:],
                                start=(dc == 0),
                                stop=(dc == ND - 1),
                            )
                    # V: [seq, head_dim] layout -> lhsT = xT chunk, rhs = wv chunk
                    for jj in range(4):
                        j = lc * 4 + jj
                        v_ps = p1ps.tile([128, DH], F32, tag="vps", bufs=2, name=f"vps{j}")
                        for dc in range(ND):
                            nc.tensor.matmul(
                                v_ps[:],
                                xts[dc][:, bass.ts(jj, 128)],
                                w_sb[5][:, bass.ts(dc, 128)],
                                start=(dc == 0),
                                stop=(dc == ND - 1),
                            )
                        nc.vector.tensor_copy(v_sb[j][:], v_ps[:])

                    # RoPE on q heads + k: out = cos*x + sin_signed*swap(x)
                    for h in range(5):
                        ps = proj_ps[h]
                        shuf = rwork.tile([128, 512], F32, tag="shuf", name=f"shuf{h}_{lc}")
                        nc.vector.stream_shuffle(shuf[:], ps[:], swap_mask)
                        qc = rwork.tile([128, 512], F32, tag="qc", name=f"qc{h}_{lc}")
                        nc.vector.tensor_mul(qc[:], ps[:], cos_sb[:, lsl])
                        qs = rwork.tile([128, 512], F32, tag="qs", name=f"qs{h}_{lc}")
                        nc.vector.tensor_mul(qs[:], shuf[:], sin_sb[:, lsl])
                        nc.vector.tensor_add(qk_rope[h][:, lsl], qc[:], qs[:])
                if debug:
                    for h in range(5):
                        nc.sync.dma_start(dbg_qk[h * 128:(h + 1) * 128, :], qk_rope[h][:])
                    for j in range(NJ):
                        nc.sync.dma_start(dbg_v[j * 128:(j + 1) * 128, :], v_sb[j][:])

            # ---------------- Phase 2: attention per head ----------------
            send = dram.tile([NCORES * HQ * DH, IS], BF16)
            krope = qk_rope[4]
            with (
                tc.tile_pool(name="expst", bufs=20) as epool,
                tc.tile_pool(name="otsb", bufs=2) as otpool,
                tc.tile_pool(name="nrm", bufs=2) as nrmpool,
                tc.tile_pool(name="p2psum", bufs=1, space="PSUM") as p2ps,
            ):
                for h in range(HQ):
                    qrope = qk_rope[h]
                    expst = []
                    for j in range(NJ):
                        et = epool.tile([128, L], BF16, tag="e", name=f"e{h}_{j}")
                        for ih in range(2):
                            s_ps = p2ps.tile(
                                [128, 1024], F32, tag="s", bufs=2, name=f"s{h}_{j}_{ih}"
                            )
                            for i2 in range(2):
                                i = ih * 2 + i2
                                nc.tensor.matmul(
                                    s_ps[:, bass.ts(i2, 512)],
                                    krope[:, bass.ts(j, 128)],
                                    qrope[:, bass.ts(i, 512)],
                                    start=True,
                                    stop=True,
                                )
                            nc.scalar.activation(
                                et[:, bass.ts(ih, 1024)],
                                s_ps[:],
                                mybir.ActivationFunctionType.Exp,
                                scale=SCALE,
                            )
                        expst.append(et)
                    if debug and h == 0:
                        for j in range(2):
                            nc.sync.dma_start(
                                dbg_e[j * 128:(j + 1) * 128, :], expst[j][:]
                            )

                    ot_sb = otpool.tile([128, L], BF16, tag="ot", name=f"ot{h}")
                    for i in range(NI):
                        isl = bass.ts(i, 512)
                        sums_ps = p2ps.tile(
                            [1, 512], F32, tag="small", bufs=2, name=f"sums{h}_{i}"
                        )
                        for j in range(NJ):
                            nc.tensor.matmul(
                                sums_ps[:],
                                ones_col[:],
                                expst[j][:, isl],
                                start=(j == 0),
                                stop=(j == NJ - 1),
                            )
                        recip = nrmpool.tile([1, 512], F32, tag="recip", name=f"rc{h}_{i}")
                        nc.vector.reciprocal(recip[:], sums_ps[:])
                        if debug and h == 0:
                            nc.sync.dma_start(dbg_rc[i:i + 1, :], recip[:])
                        rb_ps = p2ps.tile(
                            [128, 512], F32, tag="small", bufs=2, name=f"rb{h}_{i}"
                        )
                        nc.tensor.matmul(rb_ps[:], ones_row[:], recip[:], start=True, stop=True)
                        rb_sb = nrmpool.tile([128, 512], F32, tag="rb", name=f"rbs{h}_{i}")
                        nc.vector.tensor_copy(rb_sb[:], rb_ps[:])

                        ot_ps = p2ps.tile(
                            [128, 512], F32, tag="ot", bufs=2, name=f"otp{h}_{i}"
                        )
                        for j in range(NJ):
                            nc.tensor.matmul(
                                ot_ps[:],
                                v_sb[j][:],
                                expst[j][:, isl],
                                start=(j == 0),
                                stop=(j == NJ - 1),
                            )
                        nc.vector.tensor_mul(ot_sb[:, isl], ot_ps[:], rb_sb[:])

                    # scatter this head's output into the A2A send buffer:
                    # send[(c, h, p), i'] = ot_sb[p, c*IS + i']
                    for c in range(NCORES):
                        nc.sync.dma_start(
                            send[(c * HQ + h) * 128:(c * HQ + h + 1) * 128, :],
                            ot_sb[:, c * IS:(c + 1) * IS],
                        )
                    if debug and h == 0:
                        nc.sync.dma_start(dbg_ot[:], ot_sb[:])

            # ---------------- Phase 3: AllToAll + output projection ----------------
            recv = dram.tile([NCORES * HQ * DH, IS], BF16)
            if debug:
                nc.sync.dma_start(dbg_send[:], send[:])
            nc.gpsimd.collective_compute(
                "AllToAll",
                mybir.AluOpType.bypass,
                replica_groups=[list(range(NCORES))],
                ins=[send[:].opt()],
                outs=[recv[:].opt()],
            )
            if debug:
                nc.sync.dma_start(dbg_recv[:], recv[:])

            with (
                tc.tile_pool(name="og", bufs=1) as ogpool,
                tc.tile_pool(name="wos", bufs=4) as wopool,
                tc.tile_pool(name="ysb", bufs=3) as ypool,
                tc.tile_pool(name="p3psum", bufs=1, space="PSUM") as p3ps,
            ):
                og = []
                for t in range(ND):
                    gt = ogpool.tile([128, IS], BF16, name=f"og{t}")
                    nc.sync.dma_start(gt[:], recv[t * 128:(t + 1) * 128, :])
                    og.append(gt)
                if debug:
                    for t in range(ND):
                        nc.sync.dma_start(dbg_og[t * 128:(t + 1) * 128, :], og[t][:])

                for do in range(8):
                    y_ps = [
                        p3ps.tile([128, 512], F32, tag="y", bufs=4, name=f"y{do}_{ii}")
                        for ii in range(2)
                    ]
                    for hd in range(ND):
                        wo_t = wopool.tile([128, 512], BF16, tag="wo", name=f"wo{do}_{hd}")
                        nc.sync.dma_start(wo_t[:], wo[do * ND + hd, :].rearrange("(p n) -> p n", p=128))
                        for ii in range(2):
                            nc.tensor.matmul(
                                y_ps[ii][:],
                                og[hd][:, bass.ts(ii, 128)],
                                wo_t[:],
                                start=(hd == 0),
                                stop=(hd == ND - 1),
                            )
                    for ii in range(2):
                        y_sb = ypool.tile([128, 512], F32, tag="y", name=f"ysb{do}_{ii}")
                        nc.scalar.copy(y_sb[:], y_ps[ii][:])
                        nc.sync.dma_start(
                            out[ii * 128:(ii + 1) * 128, bass.ts(do, 512)], y_sb[:]
                        )

    nc.compile()
    return nc


def _rope_tables(seq_len):
    inv_freq = 1.0 / (ROPE_THETA ** (np.arange(0, DH, 2, dtype=np.float32) / DH))
    t = np.arange(seq_len, dtype=np.float32)
    freqs = t[:, None] * inv_freq[None, :]
    emb = np.concatenate([freqs, freqs], axis=-1)  # [L, DH]
    cos_e = np.cos(emb)
    sin_e = np.sin(emb)
    sign = np.where(np.arange(DH) % 2 == 0, np.float32(-1.0), np.float32(1.0))
    return cos_e.T.copy(), (sin_e * sign[None, :]).T.copy()  # [DH, L] each


def _prep_in_maps(x, wq, wk, wv, wo, seq_len):
    bf = ml_dtypes.bfloat16
    xT = np.ascontiguousarray(np.asarray(x, np.float32).reshape(L, D).T).astype(bf)
    cosT, sinT = _rope_tables(int(seq_len))

    wo_b = (
        np.asarray(wo, np.float32)
        .reshape(ND, 128, 8, 512)
        .transpose(2, 0, 1, 3)
        .reshape(8 * ND, 128 * 512)
        .astype(bf)
    )

    def head_tile(w2d):  # [D, 128] -> [128, ND*128] p-major tiling
        return (
            np.asarray(w2d, np.float32)
            .reshape(ND, 128, 128)
            .transpose(1, 0, 2)
            .reshape(128, ND * 128)
            .astype(bf)
        )

    in_maps = []
    for r in range(NCORES):
        slots = []
        for h in range(HQ):
            c = (HQ * r + h) * DH
            slots.append(head_tile(wq[:, c:c + DH]))
        slots.append(head_tile(wk[:, r * DH:(r + 1) * DH]))
        slots.append(head_tile(wv[:, r * DH:(r + 1) * DH]))
        in_maps.append(
            {
                "xT": xT,
                "wqkv": np.concatenate(slots, axis=0),
                "wo": wo_b,
                "cosT": cosT,
                "sinT": sinT,
            }
        )
    return in_maps


def kernel(x, wq, wk, wv, wo, seq_len):
    if "nc" not in _cached:
        _cached["nc"] = build_kernel()
    nc = _cached["nc"]
    in_maps = _prep_in_maps(x, wq, wk, wv, wo, seq_len)
    res = bass_utils.run_bass_kernel_spmd(
        nc, in_maps, core_ids=list(range(NCORES))
    )
    _cached["last_results"] = res
    y = np.concatenate([res.results[r]["out"] for r in range(NCORES)], axis=0)
    return y.reshape(1, L, D).astype(np.float32)


# revision 23
# speedup vs baseline: 1.2390x; 1.2390x over previous
"""Distributed GQA attention block (dense transformer) on 8 TRN2 NeuronCores.

Strategy: tensor-parallel over heads. Each core owns 4 query heads + 1 KV head
(GQA group). x^T is replicated; Q/K/V projections, RoPE, scores, softmax and
the attention output all stay in "transposed" layout (feature dim on SBUF
partitions, sequence on the free dim) so no on-device transposes are needed.
The per-core attention outputs are exchanged with a single AllToAll (each core
keeps a 256-row slice of the sequence), then each core computes its slice of
the output projection against the full (replicated) wo. Host concatenates the
8 row slices.

All matmuls run in bf16 with fp32 PSUM accumulation; softmax exp runs in fp32
on the scalar engine (no max-subtraction needed: |scores*scale| <~ 12).
"""

import numpy as np
import ml_dtypes

import concourse.bass as bass
import concourse.mybir as mybir
import concourse.tile as tile
from concourse import bacc
from concourse import bass_utils

F32 = mybir.dt.float32
BF16 = mybir.dt.bfloat16

# Problem shape (hardcoded per harness contract).
L = 2048          # sequence length
D = 4096          # model dim
DH = 128          # head dim
NHEADS = 32
NKV = 8
NCORES = 8
HQ = NHEADS // NCORES      # 4 query heads per core
ROPE_THETA = 10000.0
SCALE = DH ** -0.5

ND = D // 128              # 32 contraction chunks over model dim
NLC = L // 512             # 4 free-dim chunks of 512 over sequence
NJ = L // 128              # 16 key chunks of 128
NI = L // 512              # 4 query chunks of 512
IS = L // NCORES           # 256: per-core output row slice

_cached = {}


def build_kernel(debug=False):
    nc = bacc.Bacc(num_devices=NCORES)

    xT = nc.dram_tensor("xT", [D, L], BF16, kind="ExternalInput")
    # 6 head-slots (q0..q3, k, v), each pre-tiled to [128 partitions, 32*128]
    # where [:, k*128:(k+1)*128] is the [dim-chunk k] x [head col] block.
    wqkv = nc.dram_tensor("wqkv", [6 * 128, ND * 128], BF16, kind="ExternalInput")
    # wo pre-tiled: row (do*32+hd) holds the flattened [128, 512] block
    # wo[hd*128:(hd+1)*128, do*512:(do+1)*512].
    wo = nc.dram_tensor("wo", [8 * ND, 128 * 512], BF16, kind="ExternalInput")
    cosT = nc.dram_tensor("cosT", [128, L], F32, kind="ExternalInput")
    sinT = nc.dram_tensor("sinT", [128, L], F32, kind="ExternalInput")  # sign-folded
    out = nc.dram_tensor("out", [IS, D], F32, kind="ExternalOutput")
    if debug:
        dbg_qk = nc.dram_tensor("dbg_qk", [5 * 128, L], BF16, kind="ExternalOutput")
        dbg_v = nc.dram_tensor("dbg_v", [NJ * 128, DH], BF16, kind="ExternalOutput")
        dbg_e = nc.dram_tensor("dbg_e", [2 * 128, L], BF16, kind="ExternalOutput")
        dbg_rc = nc.dram_tensor("dbg_rc", [NI, 512], F32, kind="ExternalOutput")
        dbg_ot = nc.dram_tensor("dbg_ot", [128, L], BF16, kind="ExternalOutput")
        dbg_og = nc.dram_tensor("dbg_og", [NCORES * HQ * DH, IS], BF16, kind="ExternalOutput")
        dbg_send = nc.dram_tensor("dbg_send", [NCORES * HQ * DH, IS], BF16, kind="ExternalOutput")
        dbg_recv = nc.dram_tensor("dbg_recv", [NCORES * HQ * DH, IS], BF16, kind="ExternalOutput")

    swap_mask = []
    for i in range(16):
        swap_mask += [2 * i + 1, 2 * i]

    with tile.TileContext(nc) as tc:
        with (
            tc.tile_pool(name="const", bufs=1) as cpool,
            tc.tile_pool(name="persist", bufs=1) as ppool,
            tc.tile_pool(name="dram", bufs=1, space="DRAM") as dram,
        ):
            cos_sb = cpool.tile([128, L], F32)
            sin_sb = cpool.tile([128, L], F32)
            nc.sync.dma_start(cos_sb[:], cosT[:])
            nc.sync.dma_start(sin_sb[:], sinT[:])
            ones_bc = cpool.tile([128, 128], BF16)
            nc.vector.memset(ones_bc[:], 1.0)

            # Roped Q^T (4 heads) + roped K^T, bf16, [head_dim=128, L]
            qk_rope = [ppool.tile([128, L], BF16, name=f"qkrope{h}") for h in range(5)]
            # V in [seq, head_dim] layout: 16 chunks of [128, 128]
            v_sb = [ppool.tile([128, DH], BF16, name=f"vsb{j}") for j in range(NJ)]

            # ---------------- Phase 1: projections + rope ----------------
            with (
                tc.tile_pool(name="wq", bufs=1) as wpool,
                tc.tile_pool(name="xt", bufs=34) as xtpool,
                tc.tile_pool(name="p1psum", bufs=1, space="PSUM") as p1ps,
                tc.tile_pool(name="ropework", bufs=3) as rwork,
            ):
                w_sb = []
                for h in range(6):
                    wt = wpool.tile([128, ND * 128], BF16, name=f"w{h}")
                    nc.sync.dma_start(wt[:], wqkv[h * 128:(h + 1) * 128, :])
                    w_sb.append(wt)

                for lc in range(NLC):
                    lsl = bass.ts(lc, 512)
                    proj_ps = [
                        p1ps.tile([128, 512], F32, tag=f"proj{h}", name=f"proj{h}_{lc}")
                        for h in range(5)
                    ]
                    xts = []
                    for dc in range(ND):
                        xt_t = xtpool.tile([128, 512], BF16, tag="xt", name=f"xt{dc}_{lc}")
                        nc.sync.dma_start(xt_t[:], xT[dc * 128:(dc + 1) * 128, lsl])
                        xts.append(xt_t)
                        for h in range(5):
                            nc.tensor.matmul(
                                proj_ps[h][:],
                                w_sb[h][:, bass.ts(dc, 128)],
                                xt_t[:],
                                start=(dc == 0),
                                stop=(dc == ND - 1),
                            )
                    # V: [seq, head_dim] layout -> lhsT = xT chunk, rhs = wv chunk
                    for jj in range(4):
                        j = lc * 4 + jj
                        v_ps = p1ps.tile([128, DH], F32, tag="vps", bufs=2, name=f"vps{j}")
                        for dc in range(ND):
                            nc.tensor.matmul(
                                v_ps[:],
                                xts[dc][:, bass.ts(jj, 128)],
                                w_sb[5][:, bass.ts(dc, 128)],
                                start=(dc == 0),
                                stop=(dc == ND - 1),
                            )
                        nc.vector.tensor_copy(v_sb[j][:], v_ps[:])

                    # RoPE on q heads + k: out = cos*x + sin_signed*swap(x)
                    for h in range(5):
                        ps = proj_ps[h]
                        shuf = rwork.tile([128, 512], F32, tag="shuf", name=f"shuf{h}_{lc}")
                        nc.vector.stream_shuffle(shuf[:], ps[:], swap_mask)
                        qc = rwork.tile([128, 512], F32, tag="qc", name=f"qc{h}_{lc}")
                        nc.vector.tensor_mul(qc[:], ps[:], cos_sb[:, lsl])
                        qs = rwork.tile([128, 512], F32, tag="qs", name=f"qs{h}_{lc}")
                        nc.vector.tensor_mul(qs[:], shuf[:], sin_sb[:, lsl])
                        nc.vector.tensor_add(qk_rope[h][:, lsl], qc[:], qs[:])
                if debug:
                    for h in range(5):
                        nc.sync.dma_start(dbg_qk[h * 128:(h + 1) * 128, :], qk_rope[h][:])
                    for j in range(NJ):
                        nc.sync.dma_start(dbg_v[j * 128:(j + 1) * 128, :], v_sb[j][:])

            # ---------------- Phase 2: attention per head ----------------
            # Two A2A groups (heads 0-1, heads 2-3) so the first exchange
            # overlaps the second half of attention compute.
            sendA = dram.tile([NCORES * 2 * DH, IS], BF16)
            sendB = dram.tile([NCORES * 2 * DH, IS], BF16)
            recvA = dram.tile([NCORES * 2 * DH, IS], BF16)
            recvB = dram.tile([NCORES * 2 * DH, IS], BF16)
            krope = qk_rope[4]
            with (
                tc.tile_pool(name="expst", bufs=20) as epool,
                tc.tile_pool(name="otsb", bufs=2) as otpool,
                tc.tile_pool(name="nrm", bufs=6) as nrmpool,
                tc.tile_pool(name="p2psum", bufs=1, space="PSUM") as p2ps,
            ):
                for h in range(HQ):
                    qrope = qk_rope[h]
                    expst = []
                    for j in range(NJ):
                        et = epool.tile([128, L], BF16, tag="e", name=f"e{h}_{j}")
                        for ih in range(2):
                            s_ps = p2ps.tile(
                                [128, 1024], F32, tag="s", bufs=2, name=f"s{h}_{j}_{ih}"
                            )
                            for i2 in range(2):
                                i = ih * 2 + i2
                                nc.tensor.matmul(
                                    s_ps[:, bass.ts(i2, 512)],
                                    krope[:, bass.ts(j, 128)],
                                    qrope[:, bass.ts(i, 512)],
                                    start=True,
                                    stop=True,
                                )
                            nc.scalar.activation(
                                et[:, bass.ts(ih, 1024)],
                                s_ps[:],
                                mybir.ActivationFunctionType.Exp,
                                scale=SCALE,
                            )
                        expst.append(et)
                    if debug and h == 0:
                        for j in range(2):
                            nc.sync.dma_start(
                                dbg_e[j * 128:(j + 1) * 128, :], expst[j][:]
                            )

                    ot_sb = otpool.tile([128, L], BF16, tag="ot", name=f"ot{h}")
                    rb_sbs = []
                    for i in range(NI):
                        isl = bass.ts(i, 512)
                        # broadcast row-sums: ones[128,128]^T @ expst -> every
                        # partition holds the per-query sum
                        sums_ps = p2ps.tile(
                            [128, 512], F32, tag="small", bufs=2, name=f"sums{h}_{i}"
                        )
                        for j in range(NJ):
                            nc.tensor.matmul(
                                sums_ps[:],
                                ones_bc[:],
                                expst[j][:, isl],
                                start=(j == 0),
                                stop=(j == NJ - 1),
                            )
                        rb_sb = nrmpool.tile([128, 512], F32, tag="rb", name=f"rbs{h}_{i}")
                        nc.vector.reciprocal(rb_sb[:], sums_ps[:])
                        rb_sbs.append(rb_sb)
                        if debug and h == 0:
                            nc.sync.dma_start(dbg_rc[i:i + 1, :], rb_sb[0:1, :])
                    for i in range(NI):
                        isl = bass.ts(i, 512)
                        ot_ps = p2ps.tile(
                            [128, 512], F32, tag="ot", bufs=2, name=f"otp{h}_{i}"
                        )
                        for j in range(NJ):
                            nc.tensor.matmul(
                                ot_ps[:],
                                v_sb[j][:],
                                expst[j][:, isl],
                                start=(j == 0),
                                stop=(j == NJ - 1),
                            )
                        nc.vector.tensor_mul(ot_sb[:, isl], ot_ps[:], rb_sbs[i][:])

                    # scatter this head's output into the A2A send buffer:
                    # send[(c, h, p), i'] = ot_sb[p, c*IS + i']
                    sbuf_dram = sendA if h < 2 else sendB
                    hh = h % 2
                    for c in range(NCORES):
                        nc.sync.dma_start(
                            sbuf_dram[(c * 2 + hh) * 128:(c * 2 + hh + 1) * 128, :],
                            ot_sb[:, c * IS:(c + 1) * IS],
                        )
                    if debug and h == 0:
                        nc.sync.dma_start(dbg_ot[:], ot_sb[:])
                    if h == 1:
                        nc.gpsimd.collective_compute(
                            "AllToAll",
                            mybir.AluOpType.bypass,
                            replica_groups=[list(range(NCORES))],
                            ins=[sendA[:].opt()],
                            outs=[recvA[:].opt()],
                        )
                    elif h == 3:
                        nc.gpsimd.collective_compute(
                            "AllToAll",
                            mybir.AluOpType.bypass,
                            replica_groups=[list(range(NCORES))],
                            ins=[sendB[:].opt()],
                            outs=[recvB[:].opt()],
                        )

            # ---------------- Phase 3: output projection ----------------
            if debug:
                nc.sync.dma_start(dbg_send[0:2048, :], sendA[:])
                nc.sync.dma_start(dbg_send[2048:4096, :], sendB[:])
                nc.sync.dma_start(dbg_recv[0:2048, :], recvA[:])
                nc.sync.dma_start(dbg_recv[2048:4096, :], recvB[:])

            with (
                tc.tile_pool(name="og", bufs=1) as ogpool,
                tc.tile_pool(name="wos", bufs=4) as wopool,
                tc.tile_pool(name="ysb", bufs=3) as ypool,
                tc.tile_pool(name="p3psum", bufs=1, space="PSUM") as p3ps,
            ):
                og = []
                for t in range(ND):
                    gt = ogpool.tile([128, IS], BF16, name=f"og{t}")
                    src = recvA if t < 16 else recvB
                    tt = t % 16
                    nc.sync.dma_start(gt[:], src[tt * 128:(tt + 1) * 128, :])
                    og.append(gt)
                if debug:
                    for t in range(ND):
                        nc.sync.dma_start(dbg_og[t * 128:(t + 1) * 128, :], og[t][:])

                for do in range(8):
                    y_ps = [
                        p3ps.tile([128, 512], F32, tag="y", bufs=4, name=f"y{do}_{ii}")
                        for ii in range(2)
                    ]
                    for hd in range(ND):
                        wo_t = wopool.tile([128, 512], BF16, tag="wo", name=f"wo{do}_{hd}")
                        nc.sync.dma_start(wo_t[:], wo[do * ND + hd, :].rearrange("(p n) -> p n", p=128))
                        for ii in range(2):
                            nc.tensor.matmul(
                                y_ps[ii][:],
                                og[hd][:, bass.ts(ii, 128)],
                                wo_t[:],
                                start=(hd == 0),
                                stop=(hd == ND - 1),
                            )
                    for ii in range(2):
                        y_sb = ypool.tile([128, 512], F32, tag="y", name=f"ysb{do}_{ii}")
                        nc.scalar.copy(y_sb[:], y_ps[ii][:])
                        nc.sync.dma_start(
                            out[ii * 128:(ii + 1) * 128, bass.ts(do, 512)], y_sb[:]
                        )

    nc.compile()
    return nc


def _rope_tables(seq_len):
    inv_freq = 1.0 / (ROPE_THETA ** (np.arange(0, DH, 2, dtype=np.float32) / DH))
    t = np.arange(seq_len, dtype=np.float32)
    freqs = t[:, None] * inv_freq[None, :]
    emb = np.concatenate([freqs, freqs], axis=-1)  # [L, DH]
    cos_e = np.cos(emb)
    sin_e = np.sin(emb)
    sign = np.where(np.arange(DH) % 2 == 0, np.float32(-1.0), np.float32(1.0))
    return cos_e.T.copy(), (sin_e * sign[None, :]).T.copy()  # [DH, L] each


def _prep_in_maps(x, wq, wk, wv, wo, seq_len):
    bf = ml_dtypes.bfloat16
    xT = np.ascontiguousarray(np.asarray(x, np.float32).reshape(L, D).T).astype(bf)
    cosT, sinT = _rope_tables(int(seq_len))

    # permute the hd-contraction order to match the og tile order after the
    # two-group AllToAll: [c0h0 c0h1 c1h0 ... c7h1 | c0h2 c0h3 ... c7h3]
    perm = [c * 4 + h for c in range(NCORES) for h in (0, 1)] + [
        c * 4 + h for c in range(NCORES) for h in (2, 3)
    ]
    wo_b = (
        np.asarray(wo, np.float32)
        .reshape(ND, 128, 8, 512)
        .transpose(2, 0, 1, 3)[:, perm]
        .reshape(8 * ND, 128 * 512)
        .astype(bf)
    )

    def head_tile(w2d):  # [D, 128] -> [128, ND*128] p-major tiling
        return (
            np.asarray(w2d, np.float32)
            .reshape(ND, 128, 128)
            .transpose(1, 0, 2)
            .reshape(128, ND * 128)
            .astype(bf)
        )

    in_maps = []
    for r in range(NCORES):
        slots = []
        for h in range(HQ):
            c = (HQ * r + h) * DH
            slots.append(head_tile(wq[:, c:c + DH]))
        slots.append(head_tile(wk[:, r * DH:(r + 1) * DH]))
        slots.append(head_tile(wv[:, r * DH:(r + 1) * DH]))
        in_maps.append(
            {
                "xT": xT,
                "wqkv": np.concatenate(slots, axis=0),
                "wo": wo_b,
                "cosT": cosT,
                "sinT": sinT,
            }
        )
    return in_maps


def kernel(x, wq, wk, wv, wo, seq_len):
    if "nc" not in _cached:
        _cached["nc"] = build_kernel()
    nc = _cached["nc"]
    in_maps = _prep_in_maps(x, wq, wk, wv, wo, seq_len)
    res = bass_utils.run_bass_kernel_spmd(
        nc, in_maps, core_ids=list(range(NCORES))
    )
    _cached["last_results"] = res
    y = np.concatenate([res.results[r]["out"] for r in range(NCORES)], axis=0)
    return y.reshape(1, L, D).astype(np.float32)


# revision 24
# speedup vs baseline: 1.3250x; 1.0694x over previous
"""Distributed GQA attention block (dense transformer) on 8 TRN2 NeuronCores.

Strategy: tensor-parallel over heads. Each core owns 4 query heads + 1 KV head
(GQA group). x^T is replicated; Q/K/V projections, RoPE, scores, softmax and
the attention output all stay in "transposed" layout (feature dim on SBUF
partitions, sequence on the free dim) so no on-device transposes are needed.
The per-core attention outputs are exchanged with AllToAll collectives (each
core keeps a 256-row slice of the sequence), then each core computes its slice
of the output projection against the full (replicated, pre-tiled) wo. The host
concatenates the 8 row slices.

All matmuls run in bf16 with fp32 PSUM accumulation; softmax exp runs in fp32
on the scalar engine (no max-subtraction needed: |scores*scale| <~ 12).

Schedule notes:
- heads are software-pipelined: the sums/AV matmuls of head h-1 are emitted
  after the score matmuls of head h, so the scalar engine's exp of head h
  overlaps PE work of head h-1.
- the AllToAll is split: heads 0-2 exchange while head 3 computes; head 3's
  exchange is covered by the first 3/4 of the output-projection matmuls
  (the wo contraction order is host-permuted to put head-3 blocks last).
"""

import numpy as np
import ml_dtypes

import concourse.bass as bass
import concourse.mybir as mybir
import concourse.tile as tile
from concourse import bacc
from concourse import bass_utils

F32 = mybir.dt.float32
BF16 = mybir.dt.bfloat16

# Problem shape (hardcoded per harness contract).
L = 2048          # sequence length
D = 4096          # model dim
DH = 128          # head dim
NHEADS = 32
NKV = 8
NCORES = 8
HQ = NHEADS // NCORES      # 4 query heads per core
ROPE_THETA = 10000.0
SCALE = DH ** -0.5

ND = D // 128              # 32 contraction chunks over model dim
NLC = L // 512             # 4 free-dim chunks of 512 over sequence
NJ = L // 128              # 16 key chunks of 128
NI = L // 512              # 4 query chunks of 512
IS = L // NCORES           # 256: per-core output row slice

# AllToAll grouping: group A = heads 0..2, group B = head 3.
HA, HB = 3, 1

_cached = {}


def build_kernel(debug=False):
    nc = bacc.Bacc(num_devices=NCORES)

    xT = nc.dram_tensor("xT", [D, L], BF16, kind="ExternalInput")
    # 6 head-slots in compute order (k, q0, v, q1, q2, q3), each pre-tiled to
    # [128 partitions, 32*128]: [:, dc*128:(dc+1)*128] is dim-chunk dc.
    wqkv = nc.dram_tensor("wqkv", [6 * 128, ND * 128], BF16, kind="ExternalInput")
    # wo pre-tiled: row (do*32+t) is the flattened [128, 512] block for
    # hd-chunk perm[t] and out-column chunk do (perm = A2A arrival order).
    wo = nc.dram_tensor("wo", [8 * ND, 128 * 512], BF16, kind="ExternalInput")
    cosT = nc.dram_tensor("cosT", [128, L], F32, kind="ExternalInput")
    sinT = nc.dram_tensor("sinT", [128, L], F32, kind="ExternalInput")  # sign-folded
    out = nc.dram_tensor("out", [IS, D], F32, kind="ExternalOutput")
    if debug:
        dbg_qk = nc.dram_tensor("dbg_qk", [5 * 128, L], BF16, kind="ExternalOutput")
        dbg_v = nc.dram_tensor("dbg_v", [NJ * 128, DH], BF16, kind="ExternalOutput")
        dbg_og = nc.dram_tensor("dbg_og", [NCORES * HQ * DH, IS], BF16, kind="ExternalOutput")

    swap_mask = []
    for i in range(16):
        swap_mask += [2 * i + 1, 2 * i]

    # slot order in wqkv / processing: k, q0, v, q1, q2, q3
    SLOT_K, SLOT_Q0, SLOT_V = 0, 1, 2
    slot_of_head = [1, 3, 4, 5]  # q0..q3

    with tile.TileContext(nc) as tc:
        with (
            tc.tile_pool(name="const", bufs=1) as cpool,
            tc.tile_pool(name="persist", bufs=1) as ppool,
            tc.tile_pool(name="dram", bufs=1, space="DRAM") as dram,
        ):
            ones_bc = cpool.tile([128, 128], BF16)
            nc.vector.memset(ones_bc[:], 1.0)

            # Roped K^T + Q^T (4 heads), bf16, [head_dim=128, L]
            qk_rope = [ppool.tile([128, L], BF16, name=f"qkrope{s}") for s in range(5)]
            krope = qk_rope[0]
            qrope = [qk_rope[1], qk_rope[2], qk_rope[3], qk_rope[4]]
            rope_dst = {SLOT_K: krope, 3: qrope[1], 4: qrope[2], 5: qrope[3],
                        SLOT_Q0: qrope[0]}
            # V in [seq, head_dim] layout: 16 chunks of [128, 128]
            v_sb = [ppool.tile([128, DH], BF16, name=f"vsb{j}") for j in range(NJ)]

            # ---------------- Phase 1: projections + rope ----------------
            with (
                tc.tile_pool(name="tbl", bufs=1) as tblpool,
                tc.tile_pool(name="wq", bufs=1) as wpool,
                tc.tile_pool(name="xt", bufs=34) as xtpool,
                tc.tile_pool(name="p1psum", bufs=1, space="PSUM") as p1ps,
                tc.tile_pool(name="ropework", bufs=3) as rwork,
            ):
                cos_sb = tblpool.tile([128, L], F32)
                sin_sb = tblpool.tile([128, L], F32)
                nc.sync.dma_start(cos_sb[:], cosT[:])
                nc.sync.dma_start(sin_sb[:], sinT[:])
                w_sb = []
                for s in range(6):
                    wt = wpool.tile([128, ND * 128], BF16, name=f"w{s}")
                    nc.sync.dma_start(wt[:], wqkv[s * 128:(s + 1) * 128, :])
                    w_sb.append(wt)

                for lc in range(NLC):
                    lsl = bass.ts(lc, 512)
                    proj_slots = [SLOT_K, SLOT_Q0, 3, 4, 5]
                    proj_ps = {
                        s: p1ps.tile([128, 512], F32, tag=f"proj{s}", name=f"proj{s}_{lc}")
                        for s in proj_slots
                    }
                    xts = []
                    for dc in range(ND):
                        xt_t = xtpool.tile([128, 512], BF16, tag="xt", name=f"xt{dc}_{lc}")
                        nc.sync.dma_start(xt_t[:], xT[dc * 128:(dc + 1) * 128, lsl])
                        xts.append(xt_t)
                        for s in proj_slots:
                            nc.tensor.matmul(
                                proj_ps[s][:],
                                w_sb[s][:, bass.ts(dc, 128)],
                                xt_t[:],
                                start=(dc == 0),
                                stop=(dc == ND - 1),
                            )
                    # V: [seq, head_dim] layout -> lhsT = xT chunk, rhs = wv chunk
                    for jj in range(4):
                        j = lc * 4 + jj
                        v_ps = p1ps.tile([128, DH], F32, tag="vps", bufs=2, name=f"vps{j}")
                        for dc in range(ND):
                            nc.tensor.matmul(
                                v_ps[:],
                                xts[dc][:, bass.ts(jj, 128)],
                                w_sb[SLOT_V][:, bass.ts(dc, 128)],
                                start=(dc == 0),
                                stop=(dc == ND - 1),
                            )
                        nc.vector.tensor_copy(v_sb[j][:], v_ps[:])

                    # RoPE: out = cos*x + sin_signed*swap(x), K and Q0 first
                    for s in proj_slots:
                        ps = proj_ps[s]
                        shuf = rwork.tile([128, 512], F32, tag="shuf", name=f"sh{s}_{lc}")
                        nc.vector.stream_shuffle(shuf[:], ps[:], swap_mask)
                        qc = rwork.tile([128, 512], F32, tag="qc", name=f"qc{s}_{lc}")
                        nc.vector.tensor_mul(qc[:], ps[:], cos_sb[:, lsl])
                        qs = rwork.tile([128, 512], F32, tag="qs", name=f"qs{s}_{lc}")
                        nc.vector.tensor_mul(qs[:], shuf[:], sin_sb[:, lsl])
                        nc.vector.tensor_add(rope_dst[s][:, lsl], qc[:], qs[:])
                if debug:
                    for s, t in enumerate([krope] + qrope):
                        nc.sync.dma_start(dbg_qk[s * 128:(s + 1) * 128, :], t[:])
                    for j in range(NJ):
                        nc.sync.dma_start(dbg_v[j * 128:(j + 1) * 128, :], v_sb[j][:])

            # ---------------- Phase 2: attention, head-pipelined ----------------
            sendA = dram.tile([NCORES * HA * DH, IS], BF16)
            sendB = dram.tile([NCORES * HB * DH, IS], BF16)
            recvA = dram.tile([NCORES * HA * DH, IS], BF16)
            recvB = dram.tile([NCORES * HB * DH, IS], BF16)

            with (
                tc.tile_pool(name="expst", bufs=32) as epool,
                tc.tile_pool(name="otsb", bufs=2) as otpool,
                tc.tile_pool(name="nrm", bufs=6) as nrmpool,
                tc.tile_pool(name="p2psum", bufs=1, space="PSUM") as p2ps,
            ):
                expst_of = {}

                def s_phase(h):
                    expst = []
                    for j in range(NJ):
                        et = epool.tile([128, L], BF16, tag="e", name=f"e{h}_{j}")
                        for ih in range(2):
                            s_ps = p2ps.tile(
                                [128, 1024], F32, tag="s", bufs=2, name=f"s{h}_{j}_{ih}"
                            )
                            for i2 in range(2):
                                i = ih * 2 + i2
                                nc.tensor.matmul(
                                    s_ps[:, bass.ts(i2, 512)],
                                    krope[:, bass.ts(j, 128)],
                                    qrope[h][:, bass.ts(i, 512)],
                                    start=True,
                                    stop=True,
                                )
                            nc.scalar.activation(
                                et[:, bass.ts(ih, 1024)],
                                s_ps[:],
                                mybir.ActivationFunctionType.Exp,
                                scale=SCALE,
                            )
                        expst.append(et)
                    expst_of[h] = expst

                def av_phase(h):
                    expst = expst_of.pop(h)
                    rb_sbs = []
                    for i in range(NI):
                        isl = bass.ts(i, 512)
                        sums_ps = p2ps.tile(
                            [128, 512], F32, tag="small", bufs=2, name=f"sm{h}_{i}"
                        )
                        for j in range(NJ):
                            nc.tensor.matmul(
                                sums_ps[:],
                                ones_bc[:],
                                expst[j][:, isl],
                                start=(j == 0),
                                stop=(j == NJ - 1),
                            )
                        rb = nrmpool.tile([128, 512], F32, tag="rb", name=f"rb{h}_{i}")
                        nc.vector.reciprocal(rb[:], sums_ps[:])
                        rb_sbs.append(rb)
                    ot_sb = otpool.tile([128, L], BF16, tag="ot", name=f"ot{h}")
                    for i in range(NI):
                        isl = bass.ts(i, 512)
                        ot_ps = p2ps.tile(
                            [128, 512], F32, tag="ot", bufs=2, name=f"otp{h}_{i}"
                        )
                        for j in range(NJ):
                            nc.tensor.matmul(
                                ot_ps[:],
                                v_sb[j][:],
                                expst[j][:, isl],
                                start=(j == 0),
                                stop=(j == NJ - 1),
                            )
                        nc.vector.tensor_mul(ot_sb[:, isl], ot_ps[:], rb_sbs[i][:])
                    # scatter into A2A send buffer
                    if h < HA:
                        buf, hh, nh = sendA, h, HA
                    else:
                        buf, hh, nh = sendB, h - HA, HB
                    for c in range(NCORES):
                        nc.sync.dma_start(
                            buf[(c * nh + hh) * 128:(c * nh + hh + 1) * 128, :],
                            ot_sb[:, c * IS:(c + 1) * IS],
                        )
                    if h == HA - 1:
                        nc.gpsimd.collective_compute(
                            "AllToAll",
                            mybir.AluOpType.bypass,
                            replica_groups=[list(range(NCORES))],
                            ins=[sendA[:].opt()],
                            outs=[recvA[:].opt()],
                        )
                    elif h == HQ - 1:
                        nc.gpsimd.collective_compute(
                            "AllToAll",
                            mybir.AluOpType.bypass,
                            replica_groups=[list(range(NCORES))],
                            ins=[sendB[:].opt()],
                            outs=[recvB[:].opt()],
                        )

                for h in range(HQ):
                    s_phase(h)
                    if h > 0:
                        av_phase(h - 1)
                av_phase(HQ - 1)

            # ---------------- Phase 3: output projection ----------------
            NA = NCORES * HA  # 24 A-tiles, then 8 B-tiles
            with (
                tc.tile_pool(name="og", bufs=1) as ogpool,
                tc.tile_pool(name="wos", bufs=6) as wopool,
                tc.tile_pool(name="ysb", bufs=4) as ypool,
                tc.tile_pool(name="p3psum", bufs=1, space="PSUM") as p3ps,
            ):
                og = []
                for t in range(ND):
                    gt = ogpool.tile([128, IS], BF16, name=f"og{t}")
                    if t < NA:
                        nc.sync.dma_start(gt[:], recvA[t * 128:(t + 1) * 128, :])
                    else:
                        tt = t - NA
                        nc.sync.dma_start(gt[:], recvB[tt * 128:(tt + 1) * 128, :])
                    og.append(gt)
                if debug:
                    for t in range(ND):
                        nc.sync.dma_start(dbg_og[t * 128:(t + 1) * 128, :], og[t][:])

                for dob in range(2):
                    y_ps = [
                        [
                            p3ps.tile([128, 512], F32, tag=f"y{d2}_{ii}", name=f"y{dob}_{d2}_{ii}")
                            for ii in range(2)
                        ]
                        for d2 in range(4)
                    ]
                    for t in range(ND):
                        for d2 in range(4):
                            do = dob * 4 + d2
                            wo_t = wopool.tile([128, 512], BF16, tag="wo", name=f"wo{do}_{t}")
                            nc.sync.dma_start(
                                wo_t[:],
                                wo[do * ND + t, :].rearrange("(p n) -> p n", p=128),
                            )
                            for ii in range(2):
                                nc.tensor.matmul(
                                    y_ps[d2][ii][:],
                                    og[t][:, bass.ts(ii, 128)],
                                    wo_t[:],
                                    start=(t == 0),
                                    stop=(t == ND - 1),
                                )
                    for d2 in range(4):
                        do = dob * 4 + d2
                        for ii in range(2):
                            y_sb = ypool.tile([128, 512], F32, tag="y", name=f"ys{do}_{ii}")
                            nc.scalar.copy(y_sb[:], y_ps[d2][ii][:])
                            nc.sync.dma_start(
                                out[ii * 128:(ii + 1) * 128, bass.ts(do, 512)], y_sb[:]
                            )

    nc.compile()
    return nc


def _rope_tables(seq_len):
    inv_freq = 1.0 / (ROPE_THETA ** (np.arange(0, DH, 2, dtype=np.float32) / DH))
    t = np.arange(seq_len, dtype=np.float32)
    freqs = t[:, None] * inv_freq[None, :]
    emb = np.concatenate([freqs, freqs], axis=-1)  # [L, DH]
    cos_e = np.cos(emb)
    sin_e = np.sin(emb)
    sign = np.where(np.arange(DH) % 2 == 0, np.float32(-1.0), np.float32(1.0))
    return cos_e.T.copy(), (sin_e * sign[None, :]).T.copy()  # [DH, L] each


def _prep_in_maps(x, wq, wk, wv, wo, seq_len):
    bf = ml_dtypes.bfloat16
    xT = np.ascontiguousarray(np.asarray(x, np.float32).reshape(L, D).T).astype(bf)
    cosT, sinT = _rope_tables(int(seq_len))

    # hd-contraction order matching A2A arrival: group A (heads 0-2 of each
    # core, core-major) then group B (head 3 of each core).
    perm = [c * HQ + h for c in range(NCORES) for h in range(HA)] + [
        c * HQ + h for c in range(NCORES) for h in range(HA, HQ)
    ]
    wo_b = (
        np.asarray(wo, np.float32)
        .reshape(ND, 128, 8, 512)
        .transpose(2, 0, 1, 3)[:, perm]
        .reshape(8 * ND, 128 * 512)
        .astype(bf)
    )

    def head_tile(w2d):  # [D, 128] -> [128, ND*128] p-major tiling
        return (
            np.asarray(w2d, np.float32)
            .reshape(ND, 128, 128)
            .transpose(1, 0, 2)
            .reshape(128, ND * 128)
            .astype(bf)
        )

    in_maps = []
    for r in range(NCORES):
        q_tiles = [
            head_tile(wq[:, (HQ * r + h) * DH:(HQ * r + h + 1) * DH]) for h in range(HQ)
        ]
        # slot order: k, q0, v, q1, q2, q3
        slots = [
            head_tile(wk[:, r * DH:(r + 1) * DH]),
            q_tiles[0],
            head_tile(wv[:, r * DH:(r + 1) * DH]),
            q_tiles[1],
            q_tiles[2],
            q_tiles[3],
        ]
        in_maps.append(
            {
                "xT": xT,
                "wqkv": np.concatenate(slots, axis=0),
                "wo": wo_b,
                "cosT": cosT,
                "sinT": sinT,
            }
        )
    return in_maps


def kernel(x, wq, wk, wv, wo, seq_len):
    if "nc" not in _cached:
        _cached["nc"] = build_kernel()
    nc = _cached["nc"]
    in_maps = _prep_in_maps(x, wq, wk, wv, wo, seq_len)
    res = bass_utils.run_bass_kernel_spmd(
        nc, in_maps, core_ids=list(range(NCORES))
    )
    _cached["last_results"] = res
    y = np.concatenate([res.results[r]["out"] for r in range(NCORES)], axis=0)
    return y.reshape(1, L, D).astype(np.float32)


# revision 30
# speedup vs baseline: 1.3487x; 1.0179x over previous
"""Distributed GQA attention block (dense transformer) on 8 TRN2 NeuronCores.

Strategy: tensor-parallel over heads. Each core owns 4 query heads + 1 KV head
(GQA group). x^T is replicated; Q/K/V projections, RoPE, scores, softmax and
the attention output all stay in "transposed" layout (feature dim on SBUF
partitions, sequence on the free dim) so no on-device transposes are needed.
The per-core attention outputs are exchanged with AllToAll collectives (each
core keeps a 256-row slice of the sequence), then each core computes its slice
of the output projection against the full (replicated, pre-tiled) wo. The host
concatenates the 8 row slices.

All matmuls run in bf16 with fp32 PSUM accumulation; softmax exp runs in fp32
on the scalar engine (no max-subtraction needed: |scores*scale| <~ 12).

Schedule notes:
- heads are software-pipelined: the sums/AV matmuls of head h-1 are emitted
  after the score matmuls of head h, so the scalar engine's exp of head h
  overlaps PE work of head h-1.
- the AllToAll is split: heads 0-2 exchange while head 3 computes; head 3's
  exchange is covered by the first 3/4 of the output-projection matmuls
  (the wo contraction order is host-permuted to put head-3 blocks last).
"""

import numpy as np
import ml_dtypes

import concourse.bass as bass
import concourse.mybir as mybir
import concourse.tile as tile
from concourse import bacc
from concourse import bass_utils

F32 = mybir.dt.float32
BF16 = mybir.dt.bfloat16

# Problem shape (hardcoded per harness contract).
L = 2048          # sequence length
D = 4096          # model dim
DH = 128          # head dim
NHEADS = 32
NKV = 8
NCORES = 8
HQ = NHEADS // NCORES      # 4 query heads per core
ROPE_THETA = 10000.0
SCALE = DH ** -0.5

ND = D // 128              # 32 contraction chunks over model dim
NLC = L // 512             # 4 free-dim chunks of 512 over sequence
NJ = L // 128              # 16 key chunks of 128
NI = L // 512              # 4 query chunks of 512
IS = L // NCORES           # 256: per-core output row slice

# AllToAll groups: heads {0,1} fly during head-2 compute, {2} during head-3,
# {3} is covered by the first 3/4 of the output projection.
A2A_GROUPS = [(0, 1), (2,), (3,)]

_cached = {}


def build_kernel(debug=False):
    nc = bacc.Bacc(num_devices=NCORES)

    xT = nc.dram_tensor("xT", [D, L], BF16, kind="ExternalInput")
    # 6 head-slots in compute order (k, q0, v, q1, q2, q3), each pre-tiled to
    # [128 partitions, 32*128]: [:, dc*128:(dc+1)*128] is dim-chunk dc.
    wqkv = nc.dram_tensor("wqkv", [6 * 128, ND * 128], BF16, kind="ExternalInput")
    # wo pre-tiled: row (do*32+t) is the flattened [128, 512] block for
    # hd-chunk perm[t] and out-column chunk do (perm = A2A arrival order).
    wo = nc.dram_tensor("wo", [8 * ND, 128 * 512], BF16, kind="ExternalInput")
    cosT = nc.dram_tensor("cosT", [128, L], F32, kind="ExternalInput")
    sinT = nc.dram_tensor("sinT", [128, L], F32, kind="ExternalInput")  # sign-folded
    out = nc.dram_tensor("out", [IS, D], F32, kind="ExternalOutput")
    if debug:
        dbg_qk = nc.dram_tensor("dbg_qk", [5 * 128, L], BF16, kind="ExternalOutput")
        dbg_v = nc.dram_tensor("dbg_v", [NJ * 128, DH], BF16, kind="ExternalOutput")
        dbg_og = nc.dram_tensor("dbg_og", [NCORES * HQ * DH, IS], BF16, kind="ExternalOutput")

    swap_mask = []
    for i in range(16):
        swap_mask += [2 * i + 1, 2 * i]

    # slot order in wqkv / processing: k, q0, v, q1, q2, q3
    SLOT_K, SLOT_Q0, SLOT_V = 0, 1, 2
    slot_of_head = [1, 3, 4, 5]  # q0..q3

    with tile.TileContext(nc) as tc:
        with (
            tc.tile_pool(name="const", bufs=1) as cpool,
            tc.tile_pool(name="persist", bufs=1) as ppool,
            tc.tile_pool(name="dram", bufs=1, space="DRAM") as dram,
        ):
            ones_bc = cpool.tile([128, 128], BF16)
            nc.vector.memset(ones_bc[:], 1.0)

            # Roped K^T + Q^T (4 heads), bf16, [head_dim=128, L]
            qk_rope = [ppool.tile([128, L], BF16, name=f"qkrope{s}") for s in range(5)]
            krope = qk_rope[0]
            qrope = [qk_rope[1], qk_rope[2], qk_rope[3], qk_rope[4]]
            rope_dst = {SLOT_K: krope, 3: qrope[1], 4: qrope[2], 5: qrope[3],
                        SLOT_Q0: qrope[0]}
            # V in [seq, head_dim] layout: 16 chunks of [128, 128]
            v_sb = [ppool.tile([128, DH], BF16, name=f"vsb{j}") for j in range(NJ)]

            # ---------------- Phase 1: projections + rope ----------------
            with (
                tc.tile_pool(name="tbl", bufs=1) as tblpool,
                tc.tile_pool(name="wq", bufs=1) as wpool,
                tc.tile_pool(name="xt", bufs=34) as xtpool,
                tc.tile_pool(name="p1psum", bufs=1, space="PSUM") as p1ps,
                tc.tile_pool(name="ropework", bufs=3) as rwork,
            ):
                cos_sb = tblpool.tile([128, L], F32)
                sin_sb = tblpool.tile([128, L], F32)
                nc.sync.dma_start(cos_sb[:], cosT[:])
                nc.sync.dma_start(sin_sb[:], sinT[:])
                w_sb = []
                for s in range(6):
                    wt = wpool.tile([128, ND * 128], BF16, name=f"w{s}")
                    if s < 2:
                        # chunked so the first matmuls unblock early
                        for q in range(4):
                            nc.sync.dma_start(
                                wt[:, bass.ts(q, ND * 32)],
                                wqkv[s * 128:(s + 1) * 128, bass.ts(q, ND * 32)],
                            )
                    else:
                        nc.sync.dma_start(wt[:], wqkv[s * 128:(s + 1) * 128, :])
                    w_sb.append(wt)

                for lc in range(NLC):
                    lsl = bass.ts(lc, 512)
                    proj_slots = [SLOT_K, SLOT_Q0, 3, 4, 5]
                    proj_ps = {
                        s: p1ps.tile([128, 512], F32, tag=f"proj{s}", name=f"proj{s}_{lc}")
                        for s in proj_slots
                    }
                    xts = []
                    for dc in range(ND):
                        xt_t = xtpool.tile([128, 512], BF16, tag="xt", name=f"xt{dc}_{lc}")
                        nc.sync.dma_start(xt_t[:], xT[dc * 128:(dc + 1) * 128, lsl])
                        xts.append(xt_t)
                        for s in proj_slots:
                            nc.tensor.matmul(
                                proj_ps[s][:],
                                w_sb[s][:, bass.ts(dc, 128)],
                                xt_t[:],
                                start=(dc == 0),
                                stop=(dc == ND - 1),
                            )
                    # V: [seq, head_dim] layout -> lhsT = xT chunk, rhs = wv chunk
                    for jj in range(4):
                        j = lc * 4 + jj
                        v_ps = p1ps.tile([128, DH], F32, tag="vps", bufs=2, name=f"vps{j}")
                        for dc in range(ND):
                            nc.tensor.matmul(
                                v_ps[:],
                                xts[dc][:, bass.ts(jj, 128)],
                                w_sb[SLOT_V][:, bass.ts(dc, 128)],
                                start=(dc == 0),
                                stop=(dc == ND - 1),
                            )
                        nc.vector.tensor_copy(v_sb[j][:], v_ps[:])

                    # RoPE: out = cos*x + sin_signed*swap(x), K and Q0 first
                    for s in proj_slots:
                        ps = proj_ps[s]
                        shuf = rwork.tile([128, 512], F32, tag="shuf", name=f"sh{s}_{lc}")
                        nc.vector.stream_shuffle(shuf[:], ps[:], swap_mask)
                        qc = rwork.tile([128, 512], F32, tag="qc", name=f"qc{s}_{lc}")
                        nc.vector.tensor_mul(qc[:], ps[:], cos_sb[:, lsl])
                        qs = rwork.tile([128, 512], F32, tag="qs", name=f"qs{s}_{lc}")
                        nc.vector.tensor_mul(qs[:], shuf[:], sin_sb[:, lsl])
                        nc.vector.tensor_add(rope_dst[s][:, lsl], qc[:], qs[:])
                if debug:
                    for s, t in enumerate([krope] + qrope):
                        nc.sync.dma_start(dbg_qk[s * 128:(s + 1) * 128, :], t[:])
                    for j in range(NJ):
                        nc.sync.dma_start(dbg_v[j * 128:(j + 1) * 128, :], v_sb[j][:])

            # ---------------- Phase 2: attention, head-pipelined ----------------
            sends, recvs = [], []
            for g, grp in enumerate(A2A_GROUPS):
                sends.append(
                    dram.tile([NCORES * len(grp) * DH, IS], BF16, name=f"send{g}")
                )
                recvs.append(
                    dram.tile([NCORES * len(grp) * DH, IS], BF16, name=f"recv{g}")
                )

            with (
                tc.tile_pool(name="expst", bufs=32) as epool,
                tc.tile_pool(name="otsb", bufs=2) as otpool,
                tc.tile_pool(name="nrm", bufs=6) as nrmpool,
                tc.tile_pool(name="p2psum", bufs=1, space="PSUM") as p2ps,
            ):
                expst_of = {}

                def s_phase(h):
                    expst = []
                    for j in range(NJ):
                        et = epool.tile([128, L], BF16, tag="e", name=f"e{h}_{j}")
                        for ih in range(2):
                            s_ps = p2ps.tile(
                                [128, 1024], F32, tag="s", bufs=2, name=f"s{h}_{j}_{ih}"
                            )
                            for i2 in range(2):
                                i = ih * 2 + i2
                                nc.tensor.matmul(
                                    s_ps[:, bass.ts(i2, 512)],
                                    krope[:, bass.ts(j, 128)],
                                    qrope[h][:, bass.ts(i, 512)],
                                    start=True,
                                    stop=True,
                                )
                            nc.scalar.activation(
                                et[:, bass.ts(ih, 1024)],
                                s_ps[:],
                                mybir.ActivationFunctionType.Exp,
                                scale=SCALE,
                            )
                        expst.append(et)
                    expst_of[h] = expst

                def av_phase(h):
                    expst = expst_of.pop(h)
                    rb_sbs = []
                    for i in range(NI):
                        isl = bass.ts(i, 512)
                        sums_ps = p2ps.tile(
                            [128, 512], F32, tag="small", bufs=2, name=f"sm{h}_{i}"
                        )
                        for j in range(NJ):
                            nc.tensor.matmul(
                                sums_ps[:],
                                ones_bc[:],
                                expst[j][:, isl],
                                start=(j == 0),
                                stop=(j == NJ - 1),
                            )
                        rb = nrmpool.tile([128, 512], F32, tag="rb", name=f"rb{h}_{i}")
                        nc.vector.reciprocal(rb[:], sums_ps[:])
                        rb_sbs.append(rb)
                    ot_sb = otpool.tile([128, L], BF16, tag="ot", name=f"ot{h}")
                    for i in range(NI):
                        isl = bass.ts(i, 512)
                        ot_ps = p2ps.tile(
                            [128, 512], F32, tag="ot", bufs=2, name=f"otp{h}_{i}"
                        )
                        for j in range(NJ):
                            nc.tensor.matmul(
                                ot_ps[:],
                                v_sb[j][:],
                                expst[j][:, isl],
                                start=(j == 0),
                                stop=(j == NJ - 1),
                            )
                        nc.vector.tensor_mul(ot_sb[:, isl], ot_ps[:], rb_sbs[i][:])
                    # scatter into A2A send buffer
                    g = next(i for i, grp in enumerate(A2A_GROUPS) if h in grp)
                    grp = A2A_GROUPS[g]
                    hh, nh = grp.index(h), len(grp)
                    for c in range(NCORES):
                        nc.sync.dma_start(
                            sends[g][(c * nh + hh) * 128:(c * nh + hh + 1) * 128, :],
                            ot_sb[:, c * IS:(c + 1) * IS],
                        )
                    if h == grp[-1]:
                        nc.gpsimd.collective_compute(
                            "AllToAll",
                            mybir.AluOpType.bypass,
                            replica_groups=[list(range(NCORES))],
                            ins=[sends[g][:].opt()],
                            outs=[recvs[g][:].opt()],
                        )

                for h in range(HQ):
                    s_phase(h)
                    if h > 0:
                        av_phase(h - 1)
                av_phase(HQ - 1)

            # ---------------- Phase 3: output projection ----------------
            with (
                tc.tile_pool(name="og", bufs=1) as ogpool,
                tc.tile_pool(name="wos", bufs=8) as wopool,
                tc.tile_pool(name="ysb", bufs=4) as ypool,
                tc.tile_pool(name="p3psum", bufs=1, space="PSUM") as p3ps,
            ):
                og = []
                t = 0
                for g, grp in enumerate(A2A_GROUPS):
                    for tt in range(NCORES * len(grp)):
                        gt = ogpool.tile([128, IS], BF16, name=f"og{t}")
                        nc.sync.dma_start(gt[:], recvs[g][tt * 128:(tt + 1) * 128, :])
                        og.append(gt)
                        t += 1
                if debug:
                    for t in range(ND):
                        nc.sync.dma_start(dbg_og[t * 128:(t + 1) * 128, :], og[t][:])

                for dob in range(2):
                    y_ps = [
                        [
                            p3ps.tile([128, 512], F32, tag=f"y{d2}_{ii}", name=f"y{dob}_{d2}_{ii}")
                            for ii in range(2)
                        ]
                        for d2 in range(4)
                    ]
                    for t in range(ND):
                        for d2 in range(4):
                            do = dob * 4 + d2
                            wo_t = wopool.tile([128, 512], BF16, tag="wo", name=f"wo{do}_{t}")
                            nc.sync.dma_start(
                                wo_t[:],
                                wo[do * ND + t, :].rearrange("(p n) -> p n", p=128),
                            )
                            for ii in range(2):
                                nc.tensor.matmul(
                                    y_ps[d2][ii][:],
                                    og[t][:, bass.ts(ii, 128)],
                                    wo_t[:],
                                    start=(t == 0),
                                    stop=(t == ND - 1),
                                )
                    for d2 in range(4):
                        do = dob * 4 + d2
                        for ii in range(2):
                            y_sb = ypool.tile([128, 512], F32, tag="y", name=f"ys{do}_{ii}")
                            nc.scalar.copy(y_sb[:], y_ps[d2][ii][:])
                            nc.sync.dma_start(
                                out[ii * 128:(ii + 1) * 128, bass.ts(do, 512)], y_sb[:]
                            )

    nc.compile()
    return nc


def _rope_tables(seq_len):
    inv_freq = 1.0 / (ROPE_THETA ** (np.arange(0, DH, 2, dtype=np.float32) / DH))
    t = np.arange(seq_len, dtype=np.float32)
    freqs = t[:, None] * inv_freq[None, :]
    emb = np.concatenate([freqs, freqs], axis=-1)  # [L, DH]
    cos_e = np.cos(emb)
    sin_e = np.sin(emb)
    sign = np.where(np.arange(DH) % 2 == 0, np.float32(-1.0), np.float32(1.0))
    return cos_e.T.copy(), (sin_e * sign[None, :]).T.copy()  # [DH, L] each


def _prep_in_maps(x, wq, wk, wv, wo, seq_len):
    bf = ml_dtypes.bfloat16
    xT = np.ascontiguousarray(np.asarray(x, np.float32).reshape(L, D).T).astype(bf)
    cosT, sinT = _rope_tables(int(seq_len))

    # hd-contraction order matching A2A arrival: per group, core-major.
    perm = [
        c * HQ + h for grp in A2A_GROUPS for c in range(NCORES) for h in grp
    ]
    wo_b = (
        np.asarray(wo, np.float32)
        .reshape(ND, 128, 8, 512)
        .transpose(2, 0, 1, 3)[:, perm]
        .reshape(8 * ND, 128 * 512)
        .astype(bf)
    )

    def head_tile(w2d):  # [D, 128] -> [128, ND*128] p-major tiling
        return (
            np.asarray(w2d, np.float32)
            .reshape(ND, 128, 128)
            .transpose(1, 0, 2)
            .reshape(128, ND * 128)
            .astype(bf)
        )

    in_maps = []
    for r in range(NCORES):
        q_tiles = [
            head_tile(wq[:, (HQ * r + h) * DH:(HQ * r + h + 1) * DH]) for h in range(HQ)
        ]
        # slot order: k, q0, v, q1, q2, q3
        slots = [
            head_tile(wk[:, r * DH:(r + 1) * DH]),
            q_tiles[0],
            head_tile(wv[:, r * DH:(r + 1) * DH]),
            q_tiles[1],
            q_tiles[2],
            q_tiles[3],
        ]
        in_maps.append(
            {
                "xT": xT,
                "wqkv": np.concatenate(slots, axis=0),
                "wo": wo_b,
                "cosT": cosT,
                "sinT": sinT,
            }
        )
    return in_maps


def kernel(x, wq, wk, wv, wo, seq_len):
    if "nc" not in _cached:
        _cached["nc"] = build_kernel()
    nc = _cached["nc"]
    in_maps = _prep_in_maps(x, wq, wk, wv, wo, seq_len)
    res = bass_utils.run_bass_kernel_spmd(
        nc, in_maps, core_ids=list(range(NCORES))
    )
    _cached["last_results"] = res
    y = np.concatenate([res.results[r]["out"] for r in range(NCORES)], axis=0)
    return y.reshape(1, L, D).astype(np.float32)


# revision 33
# speedup vs baseline: 1.4326x; 1.0622x over previous
"""Distributed GQA attention block (dense transformer) on 8 TRN2 NeuronCores.

Strategy: tensor-parallel over heads. Each core owns 4 query heads + 1 KV head
(GQA group). x^T is replicated; Q/K/V projections, RoPE, scores, softmax and
the attention output all stay in "transposed" layout (feature dim on SBUF
partitions, sequence on the free dim) so no on-device transposes are needed.
The per-core attention outputs are exchanged with AllToAll collectives (each
core keeps a 256-row slice of the sequence), then each core computes its slice
of the output projection against the full (replicated, pre-tiled) wo. The host
concatenates the 8 row slices.

All matmuls run in bf16 with fp32 PSUM accumulation; softmax exp runs in fp32
on the scalar engine (no max-subtraction needed: |scores*scale| <~ 12).

Schedule notes:
- heads are software-pipelined: the sums/AV matmuls of head h-1 are emitted
  after the score matmuls of head h, so the scalar engine's exp of head h
  overlaps PE work of head h-1.
- the AllToAll is split: heads 0-2 exchange while head 3 computes; head 3's
  exchange is covered by the first 3/4 of the output-projection matmuls
  (the wo contraction order is host-permuted to put head-3 blocks last).
"""

import numpy as np
import ml_dtypes

import concourse.bass as bass
import concourse.mybir as mybir
import concourse.tile as tile
from concourse import bacc
from concourse import bass_utils

F32 = mybir.dt.float32
BF16 = mybir.dt.bfloat16

# Problem shape (hardcoded per harness contract).
L = 2048          # sequence length
D = 4096          # model dim
DH = 128          # head dim
NHEADS = 32
NKV = 8
NCORES = 8
HQ = NHEADS // NCORES      # 4 query heads per core
ROPE_THETA = 10000.0
SCALE = DH ** -0.5

ND = D // 128              # 32 contraction chunks over model dim
NLC = L // 512             # 4 free-dim chunks of 512 over sequence
NJ = L // 128              # 16 key chunks of 128
NI = L // 512              # 4 query chunks of 512
IS = L // NCORES           # 256: per-core output row slice

# AllToAll groups: heads {0,1} fly during head-2 compute, {2} during head-3,
# {3} is covered by the first 3/4 of the output projection.
A2A_GROUPS = [(0, 1), (2,), (3,)]

_cached = {}


def build_kernel(debug=False):
    nc = bacc.Bacc(num_devices=NCORES)

    xT = nc.dram_tensor("xT", [D, L], BF16, kind="ExternalInput")
    # 6 head-slots in compute order (k, q0, v, q1, q2, q3), each pre-tiled to
    # [128 partitions, 32*128]: [:, dc*128:(dc+1)*128] is dim-chunk dc.
    wqkv = nc.dram_tensor("wqkv", [6 * 128, ND * 128], BF16, kind="ExternalInput")
    # wo pre-tiled: row (do*32+t) is the flattened [128, 512] block for
    # hd-chunk perm[t] and out-column chunk do (perm = A2A arrival order).
    wo = nc.dram_tensor("wo", [8 * ND, 128 * 512], BF16, kind="ExternalInput")
    cosT = nc.dram_tensor("cosT", [128, L], F32, kind="ExternalInput")
    sinT = nc.dram_tensor("sinT", [128, L], F32, kind="ExternalInput")  # sign-folded
    out = nc.dram_tensor("out", [IS, D], F32, kind="ExternalOutput")
    if debug:
        dbg_qk = nc.dram_tensor("dbg_qk", [5 * 128, L], BF16, kind="ExternalOutput")
        dbg_v = nc.dram_tensor("dbg_v", [NJ * 128, DH], BF16, kind="ExternalOutput")
        dbg_og = nc.dram_tensor("dbg_og", [NCORES * HQ * DH, IS], BF16, kind="ExternalOutput")

    swap_mask = []
    for i in range(16):
        swap_mask += [2 * i + 1, 2 * i]

    # slot order in wqkv / processing: k, q0, v, q1, q2, q3
    SLOT_K, SLOT_Q0, SLOT_V = 0, 1, 2
    slot_of_head = [1, 3, 4, 5]  # q0..q3

    with tile.TileContext(nc) as tc:
        with (
            tc.tile_pool(name="const", bufs=1) as cpool,
            tc.tile_pool(name="persist", bufs=1) as ppool,
            tc.tile_pool(name="dram", bufs=1, space="DRAM") as dram,
        ):
            ones_bc = cpool.tile([128, 128], BF16)
            nc.vector.memset(ones_bc[:], 1.0)

            # Roped K^T + Q^T (4 heads), bf16, [head_dim=128, L]
            qk_rope = [ppool.tile([128, L], BF16, name=f"qkrope{s}") for s in range(5)]
            krope = qk_rope[0]
            qrope = [qk_rope[1], qk_rope[2], qk_rope[3], qk_rope[4]]
            rope_dst = {SLOT_K: krope, 3: qrope[1], 4: qrope[2], 5: qrope[3],
                        SLOT_Q0: qrope[0]}
            # V in [seq, head_dim] layout: 16 chunks of [128, 128]
            v_sb = [ppool.tile([128, DH], BF16, name=f"vsb{j}") for j in range(NJ)]

            # ---------------- Phase 1: projections + rope ----------------
            with (
                tc.tile_pool(name="tbl", bufs=1) as tblpool,
                tc.tile_pool(name="wq", bufs=1) as wpool,
                tc.tile_pool(name="xt", bufs=34) as xtpool,
                tc.tile_pool(name="p1psum", bufs=1, space="PSUM") as p1ps,
                tc.tile_pool(name="ropework", bufs=3) as rwork,
            ):
                # weight/table loads go through gpsimd's queue so the xt loads
                # on the sync queue aren't stuck behind them at startup
                cos_sb = tblpool.tile([128, L], F32)
                sin_sb = tblpool.tile([128, L], F32)
                nc.gpsimd.dma_start(cos_sb[:], cosT[:])
                nc.gpsimd.dma_start(sin_sb[:], sinT[:])
                w_sb = []
                for s in range(6):
                    wt = wpool.tile([128, ND * 128], BF16, name=f"w{s}")
                    if s < 2:
                        # chunked so the first matmuls unblock early
                        for q in range(4):
                            nc.gpsimd.dma_start(
                                wt[:, bass.ts(q, ND * 32)],
                                wqkv[s * 128:(s + 1) * 128, bass.ts(q, ND * 32)],
                            )
                    else:
                        nc.gpsimd.dma_start(wt[:], wqkv[s * 128:(s + 1) * 128, :])
                    w_sb.append(wt)

                for lc in range(NLC):
                    lsl = bass.ts(lc, 512)
                    proj_slots = [SLOT_K, SLOT_Q0, 3, 4, 5]
                    proj_ps = {
                        s: p1ps.tile([128, 512], F32, tag=f"proj{s}", name=f"proj{s}_{lc}")
                        for s in proj_slots
                    }
                    xts = []
                    for dc in range(ND):
                        xt_t = xtpool.tile([128, 512], BF16, tag="xt", name=f"xt{dc}_{lc}")
                        nc.sync.dma_start(xt_t[:], xT[dc * 128:(dc + 1) * 128, lsl])
                        xts.append(xt_t)
                        for s in proj_slots:
                            nc.tensor.matmul(
                                proj_ps[s][:],
                                w_sb[s][:, bass.ts(dc, 128)],
                                xt_t[:],
                                start=(dc == 0),
                                stop=(dc == ND - 1),
                            )
                    # V: [seq, head_dim] layout -> lhsT = xT chunk, rhs = wv chunk
                    for jj in range(4):
                        j = lc * 4 + jj
                        v_ps = p1ps.tile([128, DH], F32, tag="vps", bufs=2, name=f"vps{j}")
                        for dc in range(ND):
                            nc.tensor.matmul(
                                v_ps[:],
                                xts[dc][:, bass.ts(jj, 128)],
                                w_sb[SLOT_V][:, bass.ts(dc, 128)],
                                start=(dc == 0),
                                stop=(dc == ND - 1),
                            )
                        nc.vector.tensor_copy(v_sb[j][:], v_ps[:])

                    # RoPE: out = cos*x + sin_signed*swap(x), K and Q0 first
                    for s in proj_slots:
                        ps = proj_ps[s]
                        shuf = rwork.tile([128, 512], F32, tag="shuf", name=f"sh{s}_{lc}")
                        nc.vector.stream_shuffle(shuf[:], ps[:], swap_mask)
                        qc = rwork.tile([128, 512], F32, tag="qc", name=f"qc{s}_{lc}")
                        nc.vector.tensor_mul(qc[:], ps[:], cos_sb[:, lsl])
                        qs = rwork.tile([128, 512], F32, tag="qs", name=f"qs{s}_{lc}")
                        nc.vector.tensor_mul(qs[:], shuf[:], sin_sb[:, lsl])
                        nc.vector.tensor_add(rope_dst[s][:, lsl], qc[:], qs[:])
                if debug:
                    for s, t in enumerate([krope] + qrope):
                        nc.sync.dma_start(dbg_qk[s * 128:(s + 1) * 128, :], t[:])
                    for j in range(NJ):
                        nc.sync.dma_start(dbg_v[j * 128:(j + 1) * 128, :], v_sb[j][:])

            # ---------------- Phase 2: attention, head-pipelined ----------------
            sends, recvs = [], []
            for g, grp in enumerate(A2A_GROUPS):
                sends.append(
                    dram.tile([NCORES * len(grp) * DH, IS], BF16, name=f"send{g}")
                )
                recvs.append(
                    dram.tile([NCORES * len(grp) * DH, IS], BF16, name=f"recv{g}")
                )

            with (
                tc.tile_pool(name="expst", bufs=32) as epool,
                tc.tile_pool(name="otsb", bufs=2) as otpool,
                tc.tile_pool(name="nrm", bufs=6) as nrmpool,
                tc.tile_pool(name="p2psum", bufs=1, space="PSUM") as p2ps,
            ):
                expst_of = {}

                def s_phase(h):
                    expst = []
                    for j in range(NJ):
                        et = epool.tile([128, L], BF16, tag="e", name=f"e{h}_{j}")
                        for ih in range(2):
                            s_ps = p2ps.tile(
                                [128, 1024], F32, tag="s", bufs=2, name=f"s{h}_{j}_{ih}"
                            )
                            for i2 in range(2):
                                i = ih * 2 + i2
                                nc.tensor.matmul(
                                    s_ps[:, bass.ts(i2, 512)],
                                    krope[:, bass.ts(j, 128)],
                                    qrope[h][:, bass.ts(i, 512)],
                                    start=True,
                                    stop=True,
                                )
                            nc.scalar.activation(
                                et[:, bass.ts(ih, 1024)],
                                s_ps[:],
                                mybir.ActivationFunctionType.Exp,
                                scale=SCALE,
                            )
                        expst.append(et)
                    expst_of[h] = expst

                def av_phase(h):
                    expst = expst_of.pop(h)
                    rb_sbs = []
                    for i in range(NI):
                        isl = bass.ts(i, 512)
                        sums_ps = p2ps.tile(
                            [128, 512], F32, tag="small", bufs=2, name=f"sm{h}_{i}"
                        )
                        for j in range(NJ):
                            nc.tensor.matmul(
                                sums_ps[:],
                                ones_bc[:],
                                expst[j][:, isl],
                                start=(j == 0),
                                stop=(j == NJ - 1),
                            )
                        rb = nrmpool.tile([128, 512], F32, tag="rb", name=f"rb{h}_{i}")
                        nc.vector.reciprocal(rb[:], sums_ps[:])
                        rb_sbs.append(rb)
                    ot_sb = otpool.tile([128, L], BF16, tag="ot", name=f"ot{h}")
                    for i in range(NI):
                        isl = bass.ts(i, 512)
                        ot_ps = p2ps.tile(
                            [128, 512], F32, tag="ot", bufs=2, name=f"otp{h}_{i}"
                        )
                        for j in range(NJ):
                            nc.tensor.matmul(
                                ot_ps[:],
                                v_sb[j][:],
                                expst[j][:, isl],
                                start=(j == 0),
                                stop=(j == NJ - 1),
                            )
                        nc.vector.tensor_mul(ot_sb[:, isl], ot_ps[:], rb_sbs[i][:])
                    # scatter into A2A send buffer
                    g = next(i for i, grp in enumerate(A2A_GROUPS) if h in grp)
                    grp = A2A_GROUPS[g]
                    hh, nh = grp.index(h), len(grp)
                    for c in range(NCORES):
                        nc.gpsimd.dma_start(
                            sends[g][(c * nh + hh) * 128:(c * nh + hh + 1) * 128, :],
                            ot_sb[:, c * IS:(c + 1) * IS],
                        )
                    if h == grp[-1]:
                        nc.gpsimd.collective_compute(
                            "AllToAll",
                            mybir.AluOpType.bypass,
                            replica_groups=[list(range(NCORES))],
                            ins=[sends[g][:].opt()],
                            outs=[recvs[g][:].opt()],
                        )

                for h in range(HQ):
                    s_phase(h)
                    if h > 0:
                        av_phase(h - 1)
                av_phase(HQ - 1)

            # ---------------- Phase 3: output projection ----------------
            with (
                tc.tile_pool(name="og", bufs=1) as ogpool,
                tc.tile_pool(name="wos", bufs=8) as wopool,
                tc.tile_pool(name="ysb", bufs=4) as ypool,
                tc.tile_pool(name="p3psum", bufs=1, space="PSUM") as p3ps,
            ):
                # one batched load per A2A group: og_all[:, t*256:(t+1)*256]
                # holds hd-chunk t in [128, 256] layout
                og_all = ogpool.tile([128, ND * IS], BF16)
                tbase = 0
                for g, grp in enumerate(A2A_GROUPS):
                    ngt = NCORES * len(grp)
                    nc.sync.dma_start(
                        og_all[:, tbase * IS:(tbase + ngt) * IS],
                        recvs[g][:].rearrange("(t p) i -> p t i", p=128),
                    )
                    tbase += ngt
                if debug:
                    nc.sync.dma_start(
                        dbg_og[:].rearrange("(t p) i -> p t i", p=128), og_all[:]
                    )

                for dob in range(2):
                    y_ps = [
                        [
                            p3ps.tile([128, 512], F32, tag=f"y{d2}_{ii}", name=f"y{dob}_{d2}_{ii}")
                            for ii in range(2)
                        ]
                        for d2 in range(4)
                    ]
                    for tq in range(8):
                        wo_ts = []
                        for d2 in range(4):
                            do = dob * 4 + d2
                            wo_t = wopool.tile([128, 4 * 512], BF16, tag="wo", name=f"wo{do}_{tq}")
                            eng = nc.sync if d2 % 2 == 0 else nc.gpsimd
                            eng.dma_start(
                                wo_t[:],
                                wo[do * ND + tq * 4:do * ND + tq * 4 + 4, :].rearrange(
                                    "t (p n) -> p t n", p=128
                                ),
                            )
                            wo_ts.append(wo_t)
                        for tt in range(4):
                            t = tq * 4 + tt
                            for d2 in range(4):
                                for ii in range(2):
                                    nc.tensor.matmul(
                                        y_ps[d2][ii][:],
                                        og_all[:, t * IS + ii * 128:t * IS + (ii + 1) * 128],
                                        wo_ts[d2][:, bass.ts(tt, 512)],
                                        start=(t == 0),
                                        stop=(t == ND - 1),
                                    )
                    for d2 in range(4):
                        do = dob * 4 + d2
                        for ii in range(2):
                            y_sb = ypool.tile([128, 512], F32, tag="y", name=f"ys{do}_{ii}")
                            nc.scalar.copy(y_sb[:], y_ps[d2][ii][:])
                            nc.sync.dma_start(
                                out[ii * 128:(ii + 1) * 128, bass.ts(do, 512)], y_sb[:]
                            )

    nc.compile()
    return nc


def _rope_tables(seq_len):
    inv_freq = 1.0 / (ROPE_THETA ** (np.arange(0, DH, 2, dtype=np.float32) / DH))
    t = np.arange(seq_len, dtype=np.float32)
    freqs = t[:, None] * inv_freq[None, :]
    emb = np.concatenate([freqs, freqs], axis=-1)  # [L, DH]
    cos_e = np.cos(emb)
    sin_e = np.sin(emb)
    sign = np.where(np.arange(DH) % 2 == 0, np.float32(-1.0), np.float32(1.0))
    return cos_e.T.copy(), (sin_e * sign[None, :]).T.copy()  # [DH, L] each


def _prep_in_maps(x, wq, wk, wv, wo, seq_len):
    bf = ml_dtypes.bfloat16
    xT = np.ascontiguousarray(np.asarray(x, np.float32).reshape(L, D).T).astype(bf)
    cosT, sinT = _rope_tables(int(seq_len))

    # hd-contraction order matching A2A arrival: per group, core-major.
    perm = [
        c * HQ + h for grp in A2A_GROUPS for c in range(NCORES) for h in grp
    ]
    wo_b = (
        np.asarray(wo, np.float32)
        .reshape(ND, 128, 8, 512)
        .transpose(2, 0, 1, 3)[:, perm]
        .reshape(8 * ND, 128 * 512)
        .astype(bf)
    )

    def head_tile(w2d):  # [D, 128] -> [128, ND*128] p-major tiling
        return (
            np.asarray(w2d, np.float32)
            .reshape(ND, 128, 128)
            .transpose(1, 0, 2)
            .reshape(128, ND * 128)
            .astype(bf)
        )

    in_maps = []
    for r in range(NCORES):
        q_tiles = [
            head_tile(wq[:, (HQ * r + h) * DH:(HQ * r + h + 1) * DH]) for h in range(HQ)
        ]
        # slot order: k, q0, v, q1, q2, q3
        slots = [
            head_tile(wk[:, r * DH:(r + 1) * DH]),
            q_tiles[0],
            head_tile(wv[:, r * DH:(r + 1) * DH]),
            q_tiles[1],
            q_tiles[2],
            q_tiles[3],
        ]
        in_maps.append(
            {
                "xT": xT,
                "wqkv": np.concatenate(slots, axis=0),
                "wo": wo_b,
                "cosT": cosT,
                "sinT": sinT,
            }
        )
    return in_maps


def kernel(x, wq, wk, wv, wo, seq_len):
    if "nc" not in _cached:
        _cached["nc"] = build_kernel()
    nc = _cached["nc"]
    in_maps = _prep_in_maps(x, wq, wk, wv, wo, seq_len)
    res = bass_utils.run_bass_kernel_spmd(
        nc, in_maps, core_ids=list(range(NCORES))
    )
    _cached["last_results"] = res
    y = np.concatenate([res.results[r]["out"] for r in range(NCORES)], axis=0)
    return y.reshape(1, L, D).astype(np.float32)


# revision 36
# speedup vs baseline: 1.4734x; 1.0285x over previous
"""Distributed GQA attention block (dense transformer) on 8 TRN2 NeuronCores.

Strategy: tensor-parallel over heads. Each core owns 4 query heads + 1 KV head
(GQA group). x^T is replicated; Q/K/V projections, RoPE, scores, softmax and
the attention output all stay in "transposed" layout (feature dim on SBUF
partitions, sequence on the free dim) so no on-device transposes are needed.
The per-core attention outputs are exchanged with AllToAll collectives (each
core keeps a 256-row slice of the sequence), then each core computes its slice
of the output projection against the full (replicated, pre-tiled) wo. The host
concatenates the 8 row slices.

All matmuls run in bf16 with fp32 PSUM accumulation; softmax exp runs in fp32
on the scalar engine (no max-subtraction needed: |scores*scale| <~ 12).

Schedule notes:
- heads are software-pipelined: the sums/AV matmuls of head h-1 are emitted
  after the score matmuls of head h, so the scalar engine's exp of head h
  overlaps PE work of head h-1.
- the AllToAll is split: heads 0-2 exchange while head 3 computes; head 3's
  exchange is covered by the first 3/4 of the output-projection matmuls
  (the wo contraction order is host-permuted to put head-3 blocks last).
"""

import numpy as np
import ml_dtypes

import concourse.bass as bass
import concourse.mybir as mybir
import concourse.tile as tile
from concourse import bacc
from concourse import bass_utils

F32 = mybir.dt.float32
BF16 = mybir.dt.bfloat16

# Problem shape (hardcoded per harness contract).
L = 2048          # sequence length
D = 4096          # model dim
DH = 128          # head dim
NHEADS = 32
NKV = 8
NCORES = 8
HQ = NHEADS // NCORES      # 4 query heads per core
ROPE_THETA = 10000.0
SCALE = DH ** -0.5

ND = D // 128              # 32 contraction chunks over model dim
NLC = L // 512             # 4 free-dim chunks of 512 over sequence
NJ = L // 128              # 16 key chunks of 128
NI = L // 512              # 4 query chunks of 512
IS = L // NCORES           # 256: per-core output row slice

# AllToAll groups: heads {0,1} fly during head-2 compute, {2} during head-3,
# {3} is covered by the first 3/4 of the output projection.
A2A_GROUPS = [(0, 1), (2,), (3,)]

_cached = {}


def build_kernel(debug=False):
    nc = bacc.Bacc(num_devices=NCORES)

    xT = nc.dram_tensor("xT", [D, L], BF16, kind="ExternalInput")
    # 6 head-slots in compute order (k, q0, v, q1, q2, q3), each pre-tiled to
    # [128 partitions, 32*128]: [:, dc*128:(dc+1)*128] is dim-chunk dc.
    wqkv = nc.dram_tensor("wqkv", [6 * 128, ND * 128], BF16, kind="ExternalInput")
    # wo pre-tiled: row (do*32+t) is the flattened [128, 512] block for
    # hd-chunk perm[t] and out-column chunk do (perm = A2A arrival order).
    wo = nc.dram_tensor("wo", [8 * ND, 128 * 512], BF16, kind="ExternalInput")
    cosT = nc.dram_tensor("cosT", [128, L], F32, kind="ExternalInput")
    sinT = nc.dram_tensor("sinT", [128, L], F32, kind="ExternalInput")  # sign-folded
    out = nc.dram_tensor("out", [IS, D], F32, kind="ExternalOutput")
    if debug:
        dbg_qk = nc.dram_tensor("dbg_qk", [5 * 128, L], BF16, kind="ExternalOutput")
        dbg_v = nc.dram_tensor("dbg_v", [NJ * 128, DH], BF16, kind="ExternalOutput")
        dbg_og = nc.dram_tensor("dbg_og", [NCORES * HQ * DH, IS], BF16, kind="ExternalOutput")

    swap_mask = []
    for i in range(16):
        swap_mask += [2 * i + 1, 2 * i]

    # slot order in wqkv / processing: k, q0, v, q1, q2, q3
    SLOT_K, SLOT_Q0, SLOT_V = 0, 1, 2
    slot_of_head = [1, 3, 4, 5]  # q0..q3

    with tile.TileContext(nc) as tc:
        with (
            tc.tile_pool(name="const", bufs=1) as cpool,
            tc.tile_pool(name="persist", bufs=1) as ppool,
            tc.tile_pool(name="dram", bufs=1, space="DRAM") as dram,
        ):
            ones_bc = cpool.tile([128, 128], BF16)
            nc.vector.memset(ones_bc[:], 1.0)

            # Roped K^T + Q^T (4 heads), bf16, [head_dim=128, L]
            qk_rope = [ppool.tile([128, L], BF16, name=f"qkrope{s}") for s in range(5)]
            krope = qk_rope[0]
            qrope = [qk_rope[1], qk_rope[2], qk_rope[3], qk_rope[4]]
            rope_dst = {SLOT_K: krope, 3: qrope[1], 4: qrope[2], 5: qrope[3],
                        SLOT_Q0: qrope[0]}
            # V in [seq, head_dim] layout: 16 chunks of [128, 128]
            v_sb = [ppool.tile([128, DH], BF16, name=f"vsb{j}") for j in range(NJ)]

            # ---------------- Phase 1: projections + rope ----------------
            with (
                tc.tile_pool(name="tbl", bufs=1) as tblpool,
                tc.tile_pool(name="wq", bufs=1) as wpool,
                tc.tile_pool(name="xt", bufs=34) as xtpool,
                tc.tile_pool(name="p1psum", bufs=1, space="PSUM") as p1ps,
                tc.tile_pool(name="ropework", bufs=3) as rwork,
            ):
                # weight/table loads go through gpsimd's queue so the xt loads
                # on the sync queue aren't stuck behind them at startup
                cos_sb = tblpool.tile([128, L], F32)
                sin_sb = tblpool.tile([128, L], F32)
                nc.gpsimd.dma_start(cos_sb[:], cosT[:])
                nc.gpsimd.dma_start(sin_sb[:], sinT[:])
                w_sb = []
                for s in range(6):
                    wt = wpool.tile([128, ND * 128], BF16, name=f"w{s}")
                    if s < 2:
                        # chunked so the first matmuls unblock early
                        for q in range(4):
                            nc.gpsimd.dma_start(
                                wt[:, bass.ts(q, ND * 32)],
                                wqkv[s * 128:(s + 1) * 128, bass.ts(q, ND * 32)],
                            )
                    else:
                        nc.gpsimd.dma_start(wt[:], wqkv[s * 128:(s + 1) * 128, :])
                    w_sb.append(wt)

                for lc in range(NLC):
                    lsl = bass.ts(lc, 512)
                    proj_slots = [SLOT_K, SLOT_Q0, 3, 4, 5]
                    proj_ps = {
                        s: p1ps.tile([128, 512], F32, tag=f"proj{s}", name=f"proj{s}_{lc}")
                        for s in proj_slots
                    }
                    xts = []
                    for dc in range(ND):
                        xt_t = xtpool.tile([128, 512], BF16, tag="xt", name=f"xt{dc}_{lc}")
                        nc.sync.dma_start(xt_t[:], xT[dc * 128:(dc + 1) * 128, lsl])
                        xts.append(xt_t)
                        for s in proj_slots:
                            nc.tensor.matmul(
                                proj_ps[s][:],
                                w_sb[s][:, bass.ts(dc, 128)],
                                xt_t[:],
                                start=(dc == 0),
                                stop=(dc == ND - 1),
                            )
                    # V: [seq, head_dim] layout -> lhsT = xT chunk, rhs = wv chunk
                    for jj in range(4):
                        j = lc * 4 + jj
                        v_ps = p1ps.tile([128, DH], F32, tag="vps", bufs=2, name=f"vps{j}")
                        for dc in range(ND):
                            nc.tensor.matmul(
                                v_ps[:],
                                xts[dc][:, bass.ts(jj, 128)],
                                w_sb[SLOT_V][:, bass.ts(dc, 128)],
                                start=(dc == 0),
                                stop=(dc == ND - 1),
                            )
                        nc.vector.tensor_copy(v_sb[j][:], v_ps[:])

                    # RoPE: out = cos*x + sin_signed*swap(x), K and Q0 first
                    for s in proj_slots:
                        ps = proj_ps[s]
                        shuf = rwork.tile([128, 512], F32, tag="shuf", name=f"sh{s}_{lc}")
                        nc.vector.stream_shuffle(shuf[:], ps[:], swap_mask)
                        qc = rwork.tile([128, 512], F32, tag="qc", name=f"qc{s}_{lc}")
                        nc.vector.tensor_mul(qc[:], ps[:], cos_sb[:, lsl])
                        qs = rwork.tile([128, 512], F32, tag="qs", name=f"qs{s}_{lc}")
                        nc.vector.tensor_mul(qs[:], shuf[:], sin_sb[:, lsl])
                        nc.vector.tensor_add(rope_dst[s][:, lsl], qc[:], qs[:])
                if debug:
                    for s, t in enumerate([krope] + qrope):
                        nc.sync.dma_start(dbg_qk[s * 128:(s + 1) * 128, :], t[:])
                    for j in range(NJ):
                        nc.sync.dma_start(dbg_v[j * 128:(j + 1) * 128, :], v_sb[j][:])

            # ---------------- Phase 2: attention, head-pipelined ----------------
            sends, recvs = [], []
            for g, grp in enumerate(A2A_GROUPS):
                sends.append(
                    dram.tile([NCORES * len(grp) * DH, IS], BF16, name=f"send{g}")
                )
                recvs.append(
                    dram.tile([NCORES * len(grp) * DH, IS], BF16, name=f"recv{g}")
                )

            with (
                tc.tile_pool(name="expst", bufs=32) as epool,
                tc.tile_pool(name="otsb", bufs=2) as otpool,
                tc.tile_pool(name="nrm", bufs=6) as nrmpool,
                tc.tile_pool(name="p2psum", bufs=1, space="PSUM") as p2ps,
            ):
                expst_of = {}

                def s_phase(h):
                    expst = []
                    for j in range(NJ):
                        et = epool.tile([128, L], BF16, tag="e", name=f"e{h}_{j}")
                        for ih in range(2):
                            s_ps = p2ps.tile(
                                [128, 1024], F32, tag="s", bufs=2, name=f"s{h}_{j}_{ih}"
                            )
                            for i2 in range(2):
                                i = ih * 2 + i2
                                nc.tensor.matmul(
                                    s_ps[:, bass.ts(i2, 512)],
                                    krope[:, bass.ts(j, 128)],
                                    qrope[h][:, bass.ts(i, 512)],
                                    start=True,
                                    stop=True,
                                )
                            nc.scalar.activation(
                                et[:, bass.ts(ih, 1024)],
                                s_ps[:],
                                mybir.ActivationFunctionType.Exp,
                                scale=SCALE,
                            )
                        expst.append(et)
                    expst_of[h] = expst

                def av_phase(h):
                    expst = expst_of.pop(h)
                    rb_sbs = []
                    for i in range(NI):
                        isl = bass.ts(i, 512)
                        sums_ps = p2ps.tile(
                            [128, 512], F32, tag="small", bufs=2, name=f"sm{h}_{i}"
                        )
                        for j in range(NJ):
                            nc.tensor.matmul(
                                sums_ps[:],
                                ones_bc[:],
                                expst[j][:, isl],
                                start=(j == 0),
                                stop=(j == NJ - 1),
                            )
                        rb = nrmpool.tile([128, 512], F32, tag="rb", name=f"rb{h}_{i}")
                        nc.vector.reciprocal(rb[:], sums_ps[:])
                        rb_sbs.append(rb)
                    ot_sb = otpool.tile([128, L], BF16, tag="ot", name=f"ot{h}")
                    for i in range(NI):
                        isl = bass.ts(i, 512)
                        ot_ps = p2ps.tile(
                            [128, 512], F32, tag="ot", bufs=2, name=f"otp{h}_{i}"
                        )
                        for j in range(NJ):
                            nc.tensor.matmul(
                                ot_ps[:],
                                v_sb[j][:],
                                expst[j][:, isl],
                                start=(j == 0),
                                stop=(j == NJ - 1),
                            )
                        nc.vector.tensor_mul(ot_sb[:, isl], ot_ps[:], rb_sbs[i][:])
                    # scatter into A2A send buffer
                    g = next(i for i, grp in enumerate(A2A_GROUPS) if h in grp)
                    grp = A2A_GROUPS[g]
                    hh, nh = grp.index(h), len(grp)
                    for c in range(NCORES):
                        nc.gpsimd.dma_start(
                            sends[g][(c * nh + hh) * 128:(c * nh + hh + 1) * 128, :],
                            ot_sb[:, c * IS:(c + 1) * IS],
                        )
                    if h == grp[-1]:
                        nc.gpsimd.collective_compute(
                            "AllToAll",
                            mybir.AluOpType.bypass,
                            replica_groups=[list(range(NCORES))],
                            ins=[sends[g][:].opt()],
                            outs=[recvs[g][:].opt()],
                        )

                for h in range(HQ):
                    s_phase(h)
                    if h > 0:
                        av_phase(h - 1)
                av_phase(HQ - 1)

            # ---------------- Phase 3: output projection ----------------
            with (
                tc.tile_pool(name="og", bufs=1) as ogpool,
                tc.tile_pool(name="wos", bufs=10) as wopool,
                tc.tile_pool(name="ysb", bufs=4) as ypool,
                tc.tile_pool(name="p3psum", bufs=1, space="PSUM") as p3ps,
            ):
                # one batched load per A2A group: og_all[:, t*256:(t+1)*256]
                # holds hd-chunk t in [128, 256] layout
                og_all = ogpool.tile([128, ND * IS], BF16)
                tbase = 0
                for g, grp in enumerate(A2A_GROUPS):
                    ngt = NCORES * len(grp)
                    # scalar queue: keeps the sync/gpsimd wo streams free of
                    # head-of-line blocking on the late collectives
                    nc.scalar.dma_start(
                        og_all[:, tbase * IS:(tbase + ngt) * IS],
                        recvs[g][:].rearrange("(t p) i -> p t i", p=128),
                    )
                    tbase += ngt
                if debug:
                    nc.sync.dma_start(
                        dbg_og[:].rearrange("(t p) i -> p t i", p=128), og_all[:]
                    )

                for dob in range(2):
                    y_ps = [
                        [
                            p3ps.tile([128, 512], F32, tag=f"y{d2}_{ii}", name=f"y{dob}_{d2}_{ii}")
                            for ii in range(2)
                        ]
                        for d2 in range(4)
                    ]
                    for tq in range(8):
                        wo_ts = []
                        for d2 in range(4):
                            do = dob * 4 + d2
                            wo_t = wopool.tile([128, 4 * 512], BF16, tag="wo", name=f"wo{do}_{tq}")
                            eng = nc.sync if d2 % 2 == 0 else nc.gpsimd
                            eng.dma_start(
                                wo_t[:],
                                wo[do * ND + tq * 4:do * ND + tq * 4 + 4, :].rearrange(
                                    "t (p n) -> p t n", p=128
                                ),
                            )
                            wo_ts.append(wo_t)
                        for tt in range(4):
                            t = tq * 4 + tt
                            for d2 in range(4):
                                for ii in range(2):
                                    nc.tensor.matmul(
                                        y_ps[d2][ii][:],
                                        og_all[:, t * IS + ii * 128:t * IS + (ii + 1) * 128],
                                        wo_ts[d2][:, bass.ts(tt, 512)],
                                        start=(t == 0),
                                        stop=(t == ND - 1),
                                    )
                    for d2 in range(4):
                        do = dob * 4 + d2
                        for ii in range(2):
                            y_sb = ypool.tile([128, 512], F32, tag="y", name=f"ys{do}_{ii}")
                            nc.scalar.copy(y_sb[:], y_ps[d2][ii][:])
                            nc.scalar.dma_start(
                                out[ii * 128:(ii + 1) * 128, bass.ts(do, 512)], y_sb[:]
                            )

    nc.compile()
    return nc


def _rope_tables(seq_len):
    inv_freq = 1.0 / (ROPE_THETA ** (np.arange(0, DH, 2, dtype=np.float32) / DH))
    t = np.arange(seq_len, dtype=np.float32)
    freqs = t[:, None] * inv_freq[None, :]
    emb = np.concatenate([freqs, freqs], axis=-1)  # [L, DH]
    cos_e = np.cos(emb)
    sin_e = np.sin(emb)
    sign = np.where(np.arange(DH) % 2 == 0, np.float32(-1.0), np.float32(1.0))
    return cos_e.T.copy(), (sin_e * sign[None, :]).T.copy()  # [DH, L] each


def _prep_in_maps(x, wq, wk, wv, wo, seq_len):
    bf = ml_dtypes.bfloat16
    xT = np.ascontiguousarray(np.asarray(x, np.float32).reshape(L, D).T).astype(bf)
    cosT, sinT = _rope_tables(int(seq_len))

    # hd-contraction order matching A2A arrival: per group, core-major.
    perm = [
        c * HQ + h for grp in A2A_GROUPS for c in range(NCORES) for h in grp
    ]
    wo_b = (
        np.asarray(wo, np.float32)
        .reshape(ND, 128, 8, 512)
        .transpose(2, 0, 1, 3)[:, perm]
        .reshape(8 * ND, 128 * 512)
        .astype(bf)
    )

    def head_tile(w2d):  # [D, 128] -> [128, ND*128] p-major tiling
        return (
            np.asarray(w2d, np.float32)
            .reshape(ND, 128, 128)
            .transpose(1, 0, 2)
            .reshape(128, ND * 128)
            .astype(bf)
        )

    in_maps = []
    for r in range(NCORES):
        q_tiles = [
            head_tile(wq[:, (HQ * r + h) * DH:(HQ * r + h + 1) * DH]) for h in range(HQ)
        ]
        # slot order: k, q0, v, q1, q2, q3
        slots = [
            head_tile(wk[:, r * DH:(r + 1) * DH]),
            q_tiles[0],
            head_tile(wv[:, r * DH:(r + 1) * DH]),
            q_tiles[1],
            q_tiles[2],
            q_tiles[3],
        ]
        in_maps.append(
            {
                "xT": xT,
                "wqkv": np.concatenate(slots, axis=0),
                "wo": wo_b,
                "cosT": cosT,
                "sinT": sinT,
            }
        )
    return in_maps


def kernel(x, wq, wk, wv, wo, seq_len):
    if "nc" not in _cached:
        _cached["nc"] = build_kernel()
    nc = _cached["nc"]
    in_maps = _prep_in_maps(x, wq, wk, wv, wo, seq_len)
    res = bass_utils.run_bass_kernel_spmd(
        nc, in_maps, core_ids=list(range(NCORES))
    )
    _cached["last_results"] = res
    y = np.concatenate([res.results[r]["out"] for r in range(NCORES)], axis=0)
    return y.reshape(1, L, D).astype(np.float32)


# revision 38
# speedup vs baseline: 1.4747x; 1.0009x over previous
"""Distributed GQA attention block (dense transformer) on 8 TRN2 NeuronCores.

Strategy: tensor-parallel over heads. Each core owns 4 query heads + 1 KV head
(GQA group). x^T is replicated; Q/K/V projections, RoPE, scores, softmax and
the attention output all stay in "transposed" layout (feature dim on SBUF
partitions, sequence on the free dim) so no on-device transposes are needed.
The per-core attention outputs are exchanged with AllToAll collectives (each
core keeps a 256-row slice of the sequence), then each core computes its slice
of the output projection against the full (replicated, pre-tiled) wo. The host
concatenates the 8 row slices.

All matmuls run in bf16 with fp32 PSUM accumulation; softmax exp runs in fp32
on the scalar engine (no max-subtraction needed: |scores*scale| <~ 12).

Schedule notes:
- heads are software-pipelined: the sums/AV matmuls of head h-1 are emitted
  after the score matmuls of head h, so the scalar engine's exp of head h
  overlaps PE work of head h-1.
- the AllToAll is split: heads 0-2 exchange while head 3 computes; head 3's
  exchange is covered by the first 3/4 of the output-projection matmuls
  (the wo contraction order is host-permuted to put head-3 blocks last).
"""

import numpy as np
import ml_dtypes

import concourse.bass as bass
import concourse.mybir as mybir
import concourse.tile as tile
from concourse import bacc
from concourse import bass_utils

F32 = mybir.dt.float32
BF16 = mybir.dt.bfloat16

# Problem shape (hardcoded per harness contract).
L = 2048          # sequence length
D = 4096          # model dim
DH = 128          # head dim
NHEADS = 32
NKV = 8
NCORES = 8
HQ = NHEADS // NCORES      # 4 query heads per core
ROPE_THETA = 10000.0
SCALE = DH ** -0.5

ND = D // 128              # 32 contraction chunks over model dim
NLC = L // 512             # 4 free-dim chunks of 512 over sequence
NJ = L // 128              # 16 key chunks of 128
NI = L // 512              # 4 query chunks of 512
IS = L // NCORES           # 256: per-core output row slice

# AllToAll groups: heads {0,1} fly during head-2 compute, {2} during head-3,
# {3} is covered by the first 3/4 of the output projection.
A2A_GROUPS = [(0, 1), (2,), (3,)]

_cached = {}


def build_kernel(debug=False):
    nc = bacc.Bacc(num_devices=NCORES)

    xT = nc.dram_tensor("xT", [D, L], BF16, kind="ExternalInput")
    # 6 head-slots in compute order (k, q0, v, q1, q2, q3), each pre-tiled to
    # [128 partitions, 32*128]: [:, dc*128:(dc+1)*128] is dim-chunk dc.
    wqkv = nc.dram_tensor("wqkv", [6 * 128, ND * 128], BF16, kind="ExternalInput")
    # wo pre-tiled: row (do*32+t) is the flattened [128, 512] block for
    # hd-chunk perm[t] and out-column chunk do (perm = A2A arrival order).
    wo = nc.dram_tensor("wo", [8 * ND, 128 * 512], BF16, kind="ExternalInput")
    cosT = nc.dram_tensor("cosT", [128, L], F32, kind="ExternalInput")
    sinT = nc.dram_tensor("sinT", [128, L], F32, kind="ExternalInput")  # sign-folded
    out = nc.dram_tensor("out", [IS, D], F32, kind="ExternalOutput")
    if debug:
        dbg_qk = nc.dram_tensor("dbg_qk", [5 * 128, L], BF16, kind="ExternalOutput")
        dbg_v = nc.dram_tensor("dbg_v", [NJ * 128, DH], BF16, kind="ExternalOutput")
        dbg_og = nc.dram_tensor("dbg_og", [NCORES * HQ * DH, IS], BF16, kind="ExternalOutput")

    swap_mask = []
    for i in range(16):
        swap_mask += [2 * i + 1, 2 * i]

    # slot order in wqkv / processing: k, q0, v, q1, q2, q3
    SLOT_K, SLOT_Q0, SLOT_V = 0, 1, 2
    slot_of_head = [1, 3, 4, 5]  # q0..q3

    with tile.TileContext(nc) as tc:
        with (
            tc.tile_pool(name="const", bufs=1) as cpool,
            tc.tile_pool(name="persist", bufs=1) as ppool,
            tc.tile_pool(name="dram", bufs=1, space="DRAM") as dram,
        ):
            ones_bc = cpool.tile([128, 128], BF16)
            nc.vector.memset(ones_bc[:], 1.0)

            # Roped K^T + Q^T (4 heads), bf16, [head_dim=128, L]
            qk_rope = [ppool.tile([128, L], BF16, name=f"qkrope{s}") for s in range(5)]
            krope = qk_rope[0]
            qrope = [qk_rope[1], qk_rope[2], qk_rope[3], qk_rope[4]]
            rope_dst = {SLOT_K: krope, 3: qrope[1], 4: qrope[2], 5: qrope[3],
                        SLOT_Q0: qrope[0]}
            # V in [seq, head_dim] layout: 16 chunks of [128, 128]
            v_sb = [ppool.tile([128, DH], BF16, name=f"vsb{j}") for j in range(NJ)]

            # ---------------- Phase 1: projections + rope ----------------
            with (
                tc.tile_pool(name="tbl", bufs=1) as tblpool,
                tc.tile_pool(name="wq", bufs=1) as wpool,
                tc.tile_pool(name="xt", bufs=34) as xtpool,
                tc.tile_pool(name="p1psum", bufs=1, space="PSUM") as p1ps,
                tc.tile_pool(name="ropework", bufs=3) as rwork,
            ):
                # weight/table loads go through gpsimd's queue so the xt loads
                # on the sync queue aren't stuck behind them at startup
                cos_sb = tblpool.tile([128, L], F32)
                sin_sb = tblpool.tile([128, L], F32)
                nc.gpsimd.dma_start(cos_sb[:], cosT[:])
                nc.gpsimd.dma_start(sin_sb[:], sinT[:])
                w_sb = []
                for s in range(6):
                    wt = wpool.tile([128, ND * 128], BF16, name=f"w{s}")
                    if s < 2:
                        # chunked so the first matmuls unblock early
                        for q in range(4):
                            nc.gpsimd.dma_start(
                                wt[:, bass.ts(q, ND * 32)],
                                wqkv[s * 128:(s + 1) * 128, bass.ts(q, ND * 32)],
                            )
                    else:
                        nc.gpsimd.dma_start(wt[:], wqkv[s * 128:(s + 1) * 128, :])
                    w_sb.append(wt)

                for lc in range(NLC):
                    lsl = bass.ts(lc, 512)
                    proj_slots = [SLOT_K, SLOT_Q0, 3, 4, 5]
                    proj_ps = {
                        s: p1ps.tile([128, 512], F32, tag=f"proj{s}", name=f"proj{s}_{lc}")
                        for s in proj_slots
                    }
                    xts = []
                    for dc in range(ND):
                        xt_t = xtpool.tile([128, 512], BF16, tag="xt", name=f"xt{dc}_{lc}")
                        nc.sync.dma_start(xt_t[:], xT[dc * 128:(dc + 1) * 128, lsl])
                        xts.append(xt_t)
                        for s in proj_slots:
                            nc.tensor.matmul(
                                proj_ps[s][:],
                                w_sb[s][:, bass.ts(dc, 128)],
                                xt_t[:],
                                start=(dc == 0),
                                stop=(dc == ND - 1),
                            )
                    # V: [seq, head_dim] layout -> lhsT = xT chunk, rhs = wv chunk
                    for jj in range(4):
                        j = lc * 4 + jj
                        v_ps = p1ps.tile([128, DH], F32, tag="vps", bufs=2, name=f"vps{j}")
                        for dc in range(ND):
                            nc.tensor.matmul(
                                v_ps[:],
                                xts[dc][:, bass.ts(jj, 128)],
                                w_sb[SLOT_V][:, bass.ts(dc, 128)],
                                start=(dc == 0),
                                stop=(dc == ND - 1),
                            )
                        nc.vector.tensor_copy(v_sb[j][:], v_ps[:])

                    # RoPE: out = cos*x + sin_signed*swap(x), K and Q0 first
                    for s in proj_slots:
                        ps = proj_ps[s]
                        shuf = rwork.tile([128, 512], F32, tag="shuf", name=f"sh{s}_{lc}")
                        nc.vector.stream_shuffle(shuf[:], ps[:], swap_mask)
                        qc = rwork.tile([128, 512], F32, tag="qc", name=f"qc{s}_{lc}")
                        nc.vector.tensor_mul(qc[:], ps[:], cos_sb[:, lsl])
                        qs = rwork.tile([128, 512], F32, tag="qs", name=f"qs{s}_{lc}")
                        nc.vector.tensor_mul(qs[:], shuf[:], sin_sb[:, lsl])
                        nc.vector.tensor_add(rope_dst[s][:, lsl], qc[:], qs[:])
                if debug:
                    for s, t in enumerate([krope] + qrope):
                        nc.sync.dma_start(dbg_qk[s * 128:(s + 1) * 128, :], t[:])
                    for j in range(NJ):
                        nc.sync.dma_start(dbg_v[j * 128:(j + 1) * 128, :], v_sb[j][:])

            # ---------------- Phase 2: attention, head-pipelined ----------------
            sends, recvs = [], []
            for g, grp in enumerate(A2A_GROUPS):
                sends.append(
                    dram.tile([NCORES * len(grp) * DH, IS], BF16, name=f"send{g}")
                )
                recvs.append(
                    dram.tile([NCORES * len(grp) * DH, IS], BF16, name=f"recv{g}")
                )

            with (
                # phase-3 pools open FIRST so their SBUF is disjoint from the
                # attention pools: og/wo loads then never WAR-block on expst
                tc.tile_pool(name="og", bufs=1) as ogpool,
                tc.tile_pool(name="wos", bufs=10) as wopool,
                tc.tile_pool(name="ysb", bufs=4) as ypool,
                tc.tile_pool(name="expst", bufs=24) as epool,
                tc.tile_pool(name="otsb", bufs=2) as otpool,
                tc.tile_pool(name="nrm", bufs=6) as nrmpool,
            ):
              with tc.tile_pool(name="p2psum", bufs=1, space="PSUM") as p2ps:
                expst_of = {}

                def s_phase(h):
                    expst = []
                    for j in range(NJ):
                        et = epool.tile([128, L], BF16, tag="e", name=f"e{h}_{j}")
                        for ih in range(2):
                            s_ps = p2ps.tile(
                                [128, 1024], F32, tag="s", bufs=2, name=f"s{h}_{j}_{ih}"
                            )
                            for i2 in range(2):
                                i = ih * 2 + i2
                                nc.tensor.matmul(
                                    s_ps[:, bass.ts(i2, 512)],
                                    krope[:, bass.ts(j, 128)],
                                    qrope[h][:, bass.ts(i, 512)],
                                    start=True,
                                    stop=True,
                                )
                            nc.scalar.activation(
                                et[:, bass.ts(ih, 1024)],
                                s_ps[:],
                                mybir.ActivationFunctionType.Exp,
                                scale=SCALE,
                            )
                        expst.append(et)
                    expst_of[h] = expst

                def av_phase(h):
                    expst = expst_of.pop(h)
                    rb_sbs = []
                    for i in range(NI):
                        isl = bass.ts(i, 512)
                        sums_ps = p2ps.tile(
                            [128, 512], F32, tag="small", bufs=2, name=f"sm{h}_{i}"
                        )
                        for j in range(NJ):
                            nc.tensor.matmul(
                                sums_ps[:],
                                ones_bc[:],
                                expst[j][:, isl],
                                start=(j == 0),
                                stop=(j == NJ - 1),
                            )
                        rb = nrmpool.tile([128, 512], F32, tag="rb", name=f"rb{h}_{i}")
                        nc.vector.reciprocal(rb[:], sums_ps[:])
                        rb_sbs.append(rb)
                    ot_sb = otpool.tile([128, L], BF16, tag="ot", name=f"ot{h}")
                    for i in range(NI):
                        isl = bass.ts(i, 512)
                        ot_ps = p2ps.tile(
                            [128, 512], F32, tag="ot", bufs=2, name=f"otp{h}_{i}"
                        )
                        for j in range(NJ):
                            nc.tensor.matmul(
                                ot_ps[:],
                                v_sb[j][:],
                                expst[j][:, isl],
                                start=(j == 0),
                                stop=(j == NJ - 1),
                            )
                        nc.vector.tensor_mul(ot_sb[:, isl], ot_ps[:], rb_sbs[i][:])
                    # scatter into A2A send buffer
                    g = next(i for i, grp in enumerate(A2A_GROUPS) if h in grp)
                    grp = A2A_GROUPS[g]
                    hh, nh = grp.index(h), len(grp)
                    for c in range(NCORES):
                        nc.gpsimd.dma_start(
                            sends[g][(c * nh + hh) * 128:(c * nh + hh + 1) * 128, :],
                            ot_sb[:, c * IS:(c + 1) * IS],
                        )
                    if h == grp[-1]:
                        nc.gpsimd.collective_compute(
                            "AllToAll",
                            mybir.AluOpType.bypass,
                            replica_groups=[list(range(NCORES))],
                            ins=[sends[g][:].opt()],
                            outs=[recvs[g][:].opt()],
                        )

                for h in range(HQ):
                    s_phase(h)
                    if h > 0:
                        av_phase(h - 1)
                av_phase(HQ - 1)

              # ---------------- Phase 3: output projection ----------------
              with tc.tile_pool(name="p3psum", bufs=1, space="PSUM") as p3ps:
                # one batched load per A2A group: og_all[:, t*256:(t+1)*256]
                # holds hd-chunk t in [128, 256] layout
                og_all = ogpool.tile([128, ND * IS], BF16)
                tbase = 0
                for g, grp in enumerate(A2A_GROUPS):
                    ngt = NCORES * len(grp)
                    # scalar queue: keeps the sync/gpsimd wo streams free of
                    # head-of-line blocking on the late collectives
                    nc.scalar.dma_start(
                        og_all[:, tbase * IS:(tbase + ngt) * IS],
                        recvs[g][:].rearrange("(t p) i -> p t i", p=128),
                    )
                    tbase += ngt
                if debug:
                    nc.sync.dma_start(
                        dbg_og[:].rearrange("(t p) i -> p t i", p=128), og_all[:]
                    )

                for dob in range(2):
                    y_ps = [
                        [
                            p3ps.tile([128, 512], F32, tag=f"y{d2}_{ii}", name=f"y{dob}_{d2}_{ii}")
                            for ii in range(2)
                        ]
                        for d2 in range(4)
                    ]
                    for tq in range(8):
                        wo_ts = []
                        for d2 in range(4):
                            do = dob * 4 + d2
                            wo_t = wopool.tile([128, 4 * 512], BF16, tag="wo", name=f"wo{do}_{tq}")
                            eng = nc.sync if d2 % 2 == 0 else nc.gpsimd
                            eng.dma_start(
                                wo_t[:],
                                wo[do * ND + tq * 4:do * ND + tq * 4 + 4, :].rearrange(
                                    "t (p n) -> p t n", p=128
                                ),
                            )
                            wo_ts.append(wo_t)
                        for tt in range(4):
                            t = tq * 4 + tt
                            for d2 in range(4):
                                for ii in range(2):
                                    nc.tensor.matmul(
                                        y_ps[d2][ii][:],
                                        og_all[:, t * IS + ii * 128:t * IS + (ii + 1) * 128],
                                        wo_ts[d2][:, bass.ts(tt, 512)],
                                        start=(t == 0),
                                        stop=(t == ND - 1),
                                    )
                    for d2 in range(4):
                        do = dob * 4 + d2
                        for ii in range(2):
                            y_sb = ypool.tile([128, 512], F32, tag="y", name=f"ys{do}_{ii}")
                            nc.scalar.copy(y_sb[:], y_ps[d2][ii][:])
                            nc.scalar.dma_start(
                                out[ii * 128:(ii + 1) * 128, bass.ts(do, 512)], y_sb[:]
                            )

    nc.compile()
    return nc


def _rope_tables(seq_len):
    inv_freq = 1.0 / (ROPE_THETA ** (np.arange(0, DH, 2, dtype=np.float32) / DH))
    t = np.arange(seq_len, dtype=np.float32)
    freqs = t[:, None] * inv_freq[None, :]
    emb = np.concatenate([freqs, freqs], axis=-1)  # [L, DH]
    cos_e = np.cos(emb)
    sin_e = np.sin(emb)
    sign = np.where(np.arange(DH) % 2 == 0, np.float32(-1.0), np.float32(1.0))
    return cos_e.T.copy(), (sin_e * sign[None, :]).T.copy()  # [DH, L] each


def _prep_in_maps(x, wq, wk, wv, wo, seq_len):
    bf = ml_dtypes.bfloat16
    xT = np.ascontiguousarray(np.asarray(x, np.float32).reshape(L, D).T).astype(bf)
    cosT, sinT = _rope_tables(int(seq_len))

    # hd-contraction order matching A2A arrival: per group, core-major.
    perm = [
        c * HQ + h for grp in A2A_GROUPS for c in range(NCORES) for h in grp
    ]
    wo_b = (
        np.asarray(wo, np.float32)
        .reshape(ND, 128, 8, 512)
        .transpose(2, 0, 1, 3)[:, perm]
        .reshape(8 * ND, 128 * 512)
        .astype(bf)
    )

    def head_tile(w2d):  # [D, 128] -> [128, ND*128] p-major tiling
        return (
            np.asarray(w2d, np.float32)
            .reshape(ND, 128, 128)
            .transpose(1, 0, 2)
            .reshape(128, ND * 128)
            .astype(bf)
        )

    in_maps = []
    for r in range(NCORES):
        q_tiles = [
            head_tile(wq[:, (HQ * r + h) * DH:(HQ * r + h + 1) * DH]) for h in range(HQ)
        ]
        # slot order: k, q0, v, q1, q2, q3
        slots = [
            head_tile(wk[:, r * DH:(r + 1) * DH]),
            q_tiles[0],
            head_tile(wv[:, r * DH:(r + 1) * DH]),
            q_tiles[1],
            q_tiles[2],
            q_tiles[3],
        ]
        in_maps.append(
            {
                "xT": xT,
                "wqkv": np.concatenate(slots, axis=0),
                "wo": wo_b,
                "cosT": cosT,
                "sinT": sinT,
            }
        )
    return in_maps


def kernel(x, wq, wk, wv, wo, seq_len):
    if "nc" not in _cached:
        _cached["nc"] = build_kernel()
    nc = _cached["nc"]
    in_maps = _prep_in_maps(x, wq, wk, wv, wo, seq_len)
    res = bass_utils.run_bass_kernel_spmd(
        nc, in_maps, core_ids=list(range(NCORES))
    )
    _cached["last_results"] = res
    y = np.concatenate([res.results[r]["out"] for r in range(NCORES)], axis=0)
    return y.reshape(1, L, D).astype(np.float32)
